# revision 1
# baseline (speedup 1.0000x reference)
"""Trainium2 Bass kernel for ViT-style attention with decomposed relative position bias.

Problem: x(1,64,64,768) -> qkv proj -> 12-head attention with rel_pos_h/rel_pos_w
decomposed bias -> softmax -> out proj.  N=4096 tokens, hd=64.

Sharding: 8 cores = 4 head-groups (3 heads each) x 2 query-blocks (2048 queries).
Each core computes K^T/V for its 3 heads over all 4096 tokens (replicated within
the head-group pair), Q for its query block, attention, and a partial output
projection (its heads' channel slice).  Host sums the 4 head-group partials per
query block and adds proj_b.

Device layout choices:
- Scores computed transposed: [keys(partition), queries(free)] so both the QK^T
  and attn@V matmuls need no transposes anywhere.
- rel_h is folded into the scores matmul for free via contraction augmentation
  (K=64 -> 128): stationary = [scale*k ; e_kh], moving = [q ; RH^T].
- rel_w enters via exp-split: E = exp(qk+rel_h) * exp(rel_w); the second factor
  is a per-head [128, 2048] bf16 tile broadcast over key-chunks.
- Softmax denominators come free from a 65th ones-column on the V stationary.
- Normalization: 1/d via ACT Ln+Exp on the [1, q] row, broadcast to 64
  partitions with a K=1 fp32 matmul, then one DVE multiply.
- Matmuls in fp32r (full PE speed at N>=512, ~1.5e-4 rel err); E and V in bf16.
"""

import numpy as np
import ml_dtypes

NH, HD, C, H, W = 12, 64, 768, 64, 64
N = H * W            # 4096
G, QB = 4, 2         # head groups x query blocks = 8 cores
HPG = NH // G        # 3 heads per group
QL = N // QB         # 2048 queries per block
SCALE = HD ** -0.5

_prog_cache = {}


def _round_f32r(x):
    hi = x.astype(ml_dtypes.bfloat16).astype(np.float32)
    lo = (x - hi).astype(ml_dtypes.bfloat16).astype(np.float32)
    return np.ascontiguousarray(hi + lo)


def _pack6(w):
    # (768, M) -> [128, 6*M]: chunk c of the contraction at cols [c*M:(c+1)*M]
    m = w.shape[1]
    return np.ascontiguousarray(w.reshape(6, 128, m).transpose(1, 0, 2).reshape(128, 6 * m))


def _build_program(taps=False):
    import concourse.bacc as bacc
    import concourse.mybir as mybir
    import concourse.tile as tile
    from contextlib import ExitStack

    f32 = mybir.dt.float32
    f32r = mybir.dt.float32r
    bf16 = mybir.dt.bfloat16
    AF = mybir.ActivationFunctionType
    ADD = mybir.AluOpType.add

    nc = bacc.Bacc("TRN2", target_bir_lowering=False, debug=False)

    XT = nc.dram_tensor("xt", [C, N], f32r, kind="ExternalInput")
    XTQ = nc.dram_tensor("xtq", [C, QL], f32r, kind="ExternalInput")
    WA = nc.dram_tensor("wa", [128, 768], f32r, kind="ExternalInput")
    WC = nc.dram_tensor("wc", [128, 384], f32r, kind="ExternalInput")
    WVB = nc.dram_tensor("wvb", [128, 6 * 192], bf16, kind="ExternalInput")
    BVB = nc.dram_tensor("bvb", [128, 192], bf16, kind="ExternalInput")
    WQA = nc.dram_tensor("wqa", [128, 768], f32r, kind="ExternalInput")
    WQB = nc.dram_tensor("wqb", [128, 384], f32r, kind="ExternalInput")
    PW1 = nc.dram_tensor("pw1", [128, 768], f32r, kind="ExternalInput")
    PW2 = nc.dram_tensor("pw2", [64, 768], f32r, kind="ExternalInput")
    BA = nc.dram_tensor("ba", [128, 1], f32, kind="ExternalInput")
    BC_ = nc.dram_tensor("bc", [64, 1], f32, kind="ExternalInput")
    BQA = nc.dram_tensor("bqa", [128, 1], f32, kind="ExternalInput")
    BQB = nc.dram_tensor("bqb", [64, 1], f32, kind="ExternalInput")
    RHT = nc.dram_tensor("rht", [64, 32 * 64], bf16, kind="ExternalInput")
    RWT = nc.dram_tensor("rwt", [64, 64 * 64], bf16, kind="ExternalInput")
    IDKH = nc.dram_tensor("idkh", [64, N], f32r, kind="ExternalInput")
    OUT = nc.dram_tensor("out", [C, QL], f32, kind="ExternalOutput")

    VSTRIDE = HPG * 80  # 240 cols per token-tile in VN (80 per head: 64 V + 1 ones + 15 pad)
    if taps:
        TKAUG = nc.dram_tensor("t_kaug0", [128, N], f32r, kind="ExternalOutput")
        TQAUG = nc.dram_tensor("t_qaug0", [128, QL], f32r, kind="ExternalOutput")
        TEW2 = nc.dram_tensor("t_ew20", [128, QL], bf16, kind="ExternalOutput")
        TVN = nc.dram_tensor("t_vn", [128, 32 * VSTRIDE], bf16, kind="ExternalOutput")
        TE2 = nc.dram_tensor("t_e2", [128, 1024], bf16, kind="ExternalOutput")
        TOP = nc.dram_tensor("t_op", [65, 1024], f32, kind="ExternalOutput")
        TPRJ = nc.dram_tensor("t_prja", [128, QL], f32r, kind="ExternalOutput")

    with tile.TileContext(nc) as tc, ExitStack() as es:
        const = es.enter_context(tc.tile_pool(name="const", bufs=1))
        big = es.enter_context(tc.tile_pool(name="big", bufs=1))
        xp = es.enter_context(tc.tile_pool(name="xp", bufs=2))
        xbp = es.enter_context(tc.tile_pool(name="xbp", bufs=1))
        p1 = es.enter_context(tc.tile_pool(name="p1", bufs=2, space="PSUM"))
        scp = es.enter_context(tc.tile_pool(name="sc", bufs=3, space="PSUM"))
        ep = es.enter_context(tc.tile_pool(name="ep", bufs=2))
        nrm = es.enter_context(tc.tile_pool(name="nrm", bufs=1))
        ewfp = es.enter_context(tc.tile_pool(name="ewf", bufs=1))

        # ---- persistent tiles ----
        wA_t = const.tile([128, 768], f32r, tag="wA", name="wA")
        wC_t = const.tile([128, 384], f32r, tag="wC", name="wC")
        wvb_t = const.tile([128, 6 * 192], bf16, tag="wvb", name="wvb")
        bvb_t = const.tile([128, 192], bf16, tag="bvb", name="bvb")
        wqA_t = const.tile([128, 768], f32r, tag="wqA", name="wqA")
        wqB_t = const.tile([128, 384], f32r, tag="wqB", name="wqB")
        pw1_t = const.tile([128, 768], f32r, tag="pw1", name="pw1")
        pw2_t = const.tile([64, 768], f32r, tag="pw2", name="pw2")
        bA_t = const.tile([128, 1], f32, tag="bA", name="bA")
        bC_t = const.tile([64, 1], f32, tag="bC", name="bC")
        bqA_t = const.tile([128, 1], f32, tag="bqA", name="bqA")
        bqB_t = const.tile([64, 1], f32, tag="bqB", name="bqB")
        rhT_t = const.tile([64, 32 * 64], bf16, tag="rhT", name="rhT")
        rwT_t = const.tile([64, 64 * 64], bf16, tag="rwT", name="rwT")
        ones1 = const.tile([1, 64], f32, tag="ones1", name="ones1")

        for t_, d_ in [(wqA_t, WQA), (wqB_t, WQB), (bqA_t, BQA), (bqB_t, BQB),
                       (rhT_t, RHT), (rwT_t, RWT), (wA_t, WA), (wC_t, WC),
                       (wvb_t, WVB), (bvb_t, BVB), (bA_t, BA), (bC_t, BC_),
                       (pw1_t, PW1), (pw2_t, PW2)]:
            nc.sync.dma_start(t_[:], d_.ap())
        nc.vector.memset(ones1[:], 1.0)

        KAUG = [big.tile([128, N], f32r, tag=f"kaug{h}", name=f"kaug{h}") for h in range(HPG)]
        QAUG = [big.tile([128, QL], f32r, tag=f"qaug{h}", name=f"qaug{h}") for h in range(HPG)]
        
        EW2 = [big.tile([128, QL], bf16, tag=f"ew2{h}", name=f"ew2{h}") for h in range(HPG)]
        VN = big.tile([128, 32 * VSTRIDE], bf16, tag="vn", name="vn")
        PRJA = big.tile([128, QL], f32r, tag="prja", name="prja")
        PRJB = big.tile([64, QL], f32r, tag="prjb", name="prjb")

        vn3 = VN[:].rearrange("p (t c) -> p t c", c=VSTRIDE)
        for h in range(HPG):
            nc.vector.memset(vn3[:, :, 64 + 80 * h], 1.0)

        # ---- Q projection over this core's block ----
        for t in range(4):
            xc = []
            for c in range(6):
                xt_ = xp.tile([128, 512], f32r, tag=f"x{c}", name=f"x{c}")
                nc.sync.dma_start(xt_[:], XTQ.ap()[128 * c:128 * c + 128, 512 * t:512 * t + 512])
                xc.append(xt_)
            sl = slice(512 * t, 512 * t + 512)
            ps = p1.tile([128, 512], f32, tag="p1", name="p1")
            for c in range(6):
                nc.tensor.matmul(ps[:], wqA_t[:, 128 * c:128 * c + 128], xc[c][:],
                                 start=(c == 0), stop=(c == 5))
            nc.vector.tensor_scalar(QAUG[0][0:64, sl], ps[0:64, :], bqA_t[0:64, :], None, ADD)
            nc.vector.tensor_scalar(QAUG[1][0:64, sl], ps[64:128, :], bqA_t[64:128, :], None, ADD)
            ps2 = p1.tile([64, 512], f32, tag="p1", name="p1b")
            for c in range(6):
                nc.tensor.matmul(ps2[:], wqB_t[:, 64 * c:64 * c + 64], xc[c][:],
                                 start=(c == 0), stop=(c == 5))
            nc.vector.tensor_scalar(QAUG[2][0:64, sl], ps2[:], bqB_t[:], None, ADD)
        # ---- RH^T into QAUG rows 64-127; RW^T -> exp -> EW2 (bf16 matmuls) ----
        def emit_gen(h):
            qb16 = ewfp.tile([64, QL], bf16, tag="qb16", name="qb16")
            nc.vector.tensor_copy(qb16[:], QAUG[h][0:64, :])
            for i4 in range(8):
                ps = p1.tile([64, 256], f32, tag="p1", name="p1rh")
                for k in range(4):
                    i = 4 * i4 + k
                    nc.tensor.matmul(ps[:, 64 * k:64 * k + 64],
                                     rhT_t[:, 64 * i:64 * i + 64],
                                     qb16[:, 64 * i:64 * i + 64],
                                     start=True, stop=True)
                nc.vector.tensor_copy(QAUG[h][64:128, 256 * i4:256 * i4 + 256], ps[:])
            ewf = ewfp.tile([64, QL], bf16, tag="ewf", name="ewf")
            qa = qb16[:].rearrange("p (i w) -> p w i", w=64)
            ef = ewf[:].rearrange("p (i w) -> p w i", w=64)
            for w4 in range(16):
                ps = p1.tile([64, 128], f32, tag="p1", name="p1rw")
                for k in range(4):
                    w = 4 * w4 + k
                    nc.tensor.matmul(ps[:, 32 * k:32 * k + 32],
                                     rwT_t[:, 64 * w:64 * w + 64], qa[:, w, :],
                                     start=True, stop=True)
                nc.vector.tensor_copy(
                    ef[:, 4 * w4:4 * w4 + 4, :],
                    ps[:].rearrange("p (k i) -> p k i", i=32))
            nc.scalar.activation(EW2[h][0:64, :], ewf[:], AF.Exp)
            nc.vector.tensor_copy(EW2[h][64:128, :], EW2[h][0:64, :])

        emit_gen(0)
        for h in range(HPG):
            nc.sync.dma_start(KAUG[h][64:128, :], IDKH.ap())

        # ---- attention helpers ----
        O_PS = {}

        def emit_att_kc(h, qc, kc):
            q0 = 1024 * qc
            if kc == 0:
                O_PS[(h, qc)] = scp.tile([65, 1024], f32, tag="sc", name="av")
            O_ps = O_PS[(h, qc)]
            S_ps = scp.tile([128, 1024], f32, tag="sc", name="sc")
            for s in range(2):
                nc.tensor.matmul(S_ps[:, 512 * s:512 * s + 512],
                                 KAUG[h][:, 128 * kc:128 * kc + 128],
                                 QAUG[h][:, q0 + 512 * s:q0 + 512 * s + 512],
                                 start=True, stop=True)
            E1 = ep.tile([128, 1024], bf16, tag="e1", name="e1")
            nc.scalar.activation(E1[:], S_ps[:], AF.Exp)
            E2 = ep.tile([128, 1024], bf16, tag="e2", name="e2")
            nc.vector.tensor_mul(E2[:], E1[:], EW2[h][:, q0:q0 + 1024])
            if taps and h == 0 and qc == 0 and kc == 0:
                nc.sync.dma_start(TE2.ap(), E2[:])
            for s in range(2):
                nc.tensor.matmul(O_ps[:, 512 * s:512 * s + 512],
                                 VN[:, VSTRIDE * kc + 80 * h:VSTRIDE * kc + 80 * h + 65],
                                 E2[:, 512 * s:512 * s + 512],
                                 start=(kc == 0), stop=(kc == 31))

        O_SB = {}

        def emit_free(h, qc):
            O_ps = O_PS.pop((h, qc))
            if taps and h == 0 and qc == 0:
                topst = nrm.tile([65, 1024], f32, tag="topst", name="topst")
                nc.vector.tensor_copy(topst[:], O_ps[:])
                nc.sync.dma_start(TOP.ap(), topst[:])
            # copy O_ps to SBUF immediately so the PSUM slot frees early
            O_sb = const.tile([65, 1024], f32, tag="rwT", name="osb")
            nc.vector.tensor_copy(O_sb[:], O_ps[:])
            O_SB[(h, qc)] = O_sb

        def emit_norm(h, qc):
            q0 = 1024 * qc
            O_sb = O_SB.pop((h, qc))
            ln_t = nrm.tile([1, 1024], f32, tag="ln", name="ln")
            nc.scalar.activation(ln_t[:], O_sb[64:65, :], AF.Ln)
            rec_t = nrm.tile([1, 1024], f32r, tag="rec", name="rec")
            nc.scalar.activation(rec_t[:], ln_t[:], AF.Exp, scale=-1.0)
            onesr = nrm.tile([1, 64], f32r, tag="onesr", name="onesr")
            nc.vector.tensor_copy(onesr[:], ones1[:])
            B_sb = nrm.tile([64, 1024], f32, tag="bcs", name="bcs")
            for s in range(2):
                B_ps = p1.tile([64, 512], f32, tag="p1", name="bcb")
                nc.tensor.matmul(B_ps[:], onesr[:],
                                 rec_t[:, 512 * s:512 * s + 512], start=True, stop=True)
                nc.vector.tensor_copy(B_sb[:, 512 * s:512 * s + 512], B_ps[:])
            dst = PRJA[64 * h:64 * h + 64, q0:q0 + 1024] if h < 2 else PRJB[0:64, q0:q0 + 1024]
            nc.vector.tensor_mul(dst, O_sb[0:64, :], B_sb[:])

        # ---- K projection + V natural layout, streaming all tokens;
        #      head-0/qc-0 attention chases the stream chunk by chunk ----
        for t in range(8):
            xc = []
            xb = []
            for c in range(6):
                xt_ = xp.tile([128, 512], f32r, tag=f"x{c}", name=f"x{c}")
                nc.sync.dma_start(xt_[:], XT.ap()[128 * c:128 * c + 128, 512 * t:512 * t + 512])
                xc.append(xt_)
                xb_ = xbp.tile([128, 512], bf16, tag=f"xb{c}", name=f"xb{c}")
                nc.gpsimd.tensor_copy(xb_[:], xt_[:])
                xb.append(xb_)
            sl = slice(512 * t, 512 * t + 512)
            ps = p1.tile([128, 512], f32, tag="p1", name="p1k")
            for c in range(6):
                nc.tensor.matmul(ps[:], wA_t[:, 128 * c:128 * c + 128], xc[c][:],
                                 start=(c == 0), stop=(c == 5))
            nc.vector.tensor_scalar(KAUG[0][0:64, sl], ps[0:64, :], bA_t[0:64, :], None, ADD)
            nc.vector.tensor_scalar(KAUG[1][0:64, sl], ps[64:128, :], bA_t[64:128, :], None, ADD)
            ps2 = p1.tile([64, 512], f32, tag="p1", name="p1k2")
            for c in range(6):
                nc.tensor.matmul(ps2[:], wC_t[:, 64 * c:64 * c + 64], xc[c][:],
                                 start=(c == 0), stop=(c == 5))
            nc.vector.tensor_scalar(KAUG[2][0:64, sl], ps2[:], bC_t[:], None, ADD)
            for s in range(4):
                tt = 4 * t + s
                pv = p1.tile([128, 192], f32, tag="p1", name="p1v")
                for c in range(6):
                    nc.tensor.matmul(pv[:], xb[c][:, 128 * s:128 * s + 128],
                                     wvb_t[:, 192 * c:192 * c + 192],
                                     start=(c == 0), stop=(c == 5))
                vdst = VN[:, VSTRIDE * tt:VSTRIDE * tt + VSTRIDE].rearrange(
                    "p (h c) -> p h c", c=80)[:, :, 0:64]
                nc.vector.tensor_tensor(
                    vdst, pv[:].rearrange("p (h c) -> p h c", c=64),
                    bvb_t[:].rearrange("p (h c) -> p h c", c=64), ADD)
            for kc in range(4 * t, 4 * t + 4):
                emit_att_kc(0, 0, kc)
        emit_gen(1)
        emit_gen(2)
        def emit_proj(qc, m, s, pool, tag):
            q0 = 1024 * qc + 512 * s
            ps = pool.tile([128, 512], f32, tag=tag, name="po")
            nc.tensor.matmul(ps[:], pw1_t[:, 128 * m:128 * m + 128],
                             PRJA[:, q0:q0 + 512], start=True, stop=False)
            nc.tensor.matmul(ps[:], pw2_t[:, 128 * m:128 * m + 128],
                             PRJB[:, q0:q0 + 512], start=False, stop=True)
            ost = big.tile([128, 512], f32, tag=f"ew2{(2 * m + s) % 3}", name="ost")
            if (2 * m + s) % 2 == 0:
                nc.vector.tensor_copy(ost[:], ps[:])
            else:
                nc.scalar.copy(ost[:], ps[:])
            nc.sync.dma_start(OUT.ap()[128 * m:128 * m + 128, q0:q0 + 512], ost[:])

        emit_free(0, 0)
        prev = (0, 0)
        for h, qc in [(0, 1), (1, 0), (1, 1), (2, 0), (2, 1)]:
            for kc in range(32):
                emit_att_kc(h, qc, kc)
                if kc == 3 and prev is not None:
                    emit_norm(*prev)
                if h == 2 and qc == 1 and kc in (8, 12, 16, 20) and prev == (2, 0):
                    # hide qc0 output projection under the last attention segment
                    for m in range((kc - 8) * 6 // 16, (kc - 4) * 6 // 16):
                        for s in range(2):
                            emit_proj(0, m, s, p1, "p1")
            emit_free(h, qc)
            prev = (h, qc)
        emit_norm(2, 1)

        if taps:
            nc.sync.dma_start(TKAUG.ap(), KAUG[0][:])
            nc.sync.dma_start(TQAUG.ap(), QAUG[0][:])
            nc.sync.dma_start(TEW2.ap(), EW2[0][:])
            nc.sync.dma_start(TVN.ap(), VN[:])
            nc.sync.dma_start(TPRJ.ap(), PRJA[:])

        # ---- output projection qc1 (qc0 was hidden under the last segment) ----
        for m in range(6):
            for s in range(2):
                emit_proj(1, m, s, scp, "sc")

    nc.compile()
    return nc


def _host_inputs(x, qkv_w, qkv_b, proj_w, rel_pos_h, rel_pos_w):
    """Build the 8 per-core input maps."""
    xmat = np.ascontiguousarray(x.reshape(N, C))
    xT = _round_f32r(xmat.T.astype(np.float32))

    idx = np.arange(64)[:, None] - np.arange(64)[None, :] + 63
    rh_g = rel_pos_h[idx]            # (h, kh, c)
    rw_g = rel_pos_w[idx]            # (w, kw, c)
    rwT = np.ascontiguousarray(rw_g.transpose(2, 0, 1).reshape(64, 64 * 64)).astype(ml_dtypes.bfloat16)
    idkh = _round_f32r(
        (np.arange(64)[:, None] == (np.arange(N)[None, :] // 64)).astype(np.float32))

    in_maps = []
    for core in range(8):
        g, j = core // QB, core % QB
        cs = slice(192 * g, 192 * g + 192)
        wq = qkv_w[:, 0 * C:1 * C][:, cs]
        wk = qkv_w[:, 1 * C:2 * C][:, cs] * SCALE
        wv = qkv_w[:, 2 * C:3 * C][:, cs]
        bq = qkv_b[0 * C:1 * C][cs]
        bk = qkv_b[1 * C:2 * C][cs] * SCALE
        bv = qkv_b[2 * C:3 * C][cs]

        h0 = 32 * j
        rhT = np.ascontiguousarray(rh_g[h0:h0 + 32].transpose(2, 0, 1).reshape(64, 32 * 64)).astype(ml_dtypes.bfloat16)

        m = {
            "xt": xT,
            "xtq": np.ascontiguousarray(xT[:, QL * j:QL * j + QL]),
            "wa": _round_f32r(_pack6(wk[:, 0:128])),
            "wc": _round_f32r(_pack6(wk[:, 128:192])),
            "wvb": _pack6(wv).astype(ml_dtypes.bfloat16),
            "bvb": np.ascontiguousarray(
                np.broadcast_to(bv[None, :], (128, 192))).astype(ml_dtypes.bfloat16),
            "wqa": _round_f32r(_pack6(wq[:, 0:128])),
            "wqb": _round_f32r(_pack6(wq[:, 128:192])),
            "pw1": _round_f32r(proj_w[cs][0:128, :]),
            "pw2": _round_f32r(proj_w[cs][128:192, :]),
            "ba": np.ascontiguousarray(bk[0:128, None].astype(np.float32)),
            "bc": np.ascontiguousarray(bk[128:192, None].astype(np.float32)),
            "bqa": np.ascontiguousarray(bq[0:128, None].astype(np.float32)),
            "bqb": np.ascontiguousarray(bq[128:192, None].astype(np.float32)),
            "rht": rhT,
            "rwt": rwT,
            "idkh": idkh,
        }
        in_maps.append(m)
    return in_maps


def kernel(x, qkv_w, qkv_b, proj_w, proj_b, rel_pos_h, rel_pos_w):
    from concourse.bass_utils import run_bass_kernel_spmd

    x = np.asarray(x, dtype=np.float32)
    qkv_w = np.asarray(qkv_w, dtype=np.float32)
    qkv_b = np.asarray(qkv_b, dtype=np.float32)
    proj_w = np.asarray(proj_w, dtype=np.float32)
    proj_b = np.asarray(proj_b, dtype=np.float32)
    rel_pos_h = np.asarray(rel_pos_h, dtype=np.float32)
    rel_pos_w = np.asarray(rel_pos_w, dtype=np.float32)

    if "nc" not in _prog_cache:
        _prog_cache["nc"] = _build_program()
    nc = _prog_cache["nc"]

    in_maps = _host_inputs(x, qkv_w, qkv_b, proj_w, rel_pos_h, rel_pos_w)
    res = run_bass_kernel_spmd(nc, in_maps, core_ids=list(range(8)))

    out = np.zeros((N, C), dtype=np.float32)
    for core in range(8):
        g, j = core // QB, core % QB
        out[QL * j:QL * j + QL, :] += res.results[core]["out"].T
    out += proj_b[None, :]
    return out.reshape(1, H, W, C).astype(np.float32)



# revision 6
# speedup vs baseline: 1.2071x; 1.2071x over previous
"""Trainium2 Bass kernel for ViT-style attention with decomposed relative
position bias (fp8 DoubleRow edition).

Problem: x(1,64,64,768) -> qkv proj -> 12-head attention with rel_pos_h/w
decomposed bias -> softmax -> out proj.  N=4096 tokens, hd=64.

Sharding: 8 cores = 4 head-groups (3 heads) x 2 query-blocks (2048 queries).

Per-core design (all matmul-heavy paths in fp8 DoubleRow at 0.5 cyc/row):
- Scores: ONE fp8-DR matmul per 128-key chunk folds everything:
    stationary slots  p0-63:(K,K)dup   p64-127:(IDKH,IDKW) one-hots
    moving   slots  p0-63:(Qhi,Qlo)  p64-127:(RH,RW)
  => S = K*(Qhi+Qlo) + rel_h + rel_w, K 8-bit/Q 12-bit, pre-scaled by
  SK*SQ=64 (fp8 range), un-scaled inside exp.
- exp split across engines: even kc-pairs on ACT (true exp -> fp8 E, AV is a
  fp8-DR pair matmul), odd pairs on DVE via Schraudolph bit-trick exp
  (int16 = S*a+b, bitcast bf16; AV is bf16 matmul on fp8 V stationary).
- Softmax denominators from a ones-column in the V stationary; 1/d via
  reciprocal_approx_fast (DVE), broadcast on GPSIMD, fused normalize-mul
  into fp8 PRJ tile (DVE).
- K/Q/V generation in bf16 (exact-ish); K/V/rel converts on ACT
  (Identity/Copy activations share the exp table set -> no table reloads);
  output projection as fp8-DR (hi/lo weights), DMA'd straight from PSUM.
- Head-alternating partition layout (h0:p0-63, h1:p64-127, h2:p0-63 for
  K/Q; one-hots/rel on the complement) so gen-PSUM rows map 1:1.
"""

import numpy as np
import ml_dtypes

NH, HD, C, H, W = 12, 64, 768, 64, 64
N = H * W            # 4096
G, QB = 4, 2         # head groups x query blocks = 8 cores
HPG = NH // G        # 3 heads per group
QL = N // QB         # 2048 queries per block
SCALE = HD ** -0.5

SK, SQ, SREL, SV = 16.0, 4.0, 64.0, 16.0
SEXP = SK * SQ       # scores arrive in PSUM scaled by 64
SP = 256.0           # PRJ tile scale (PRJ = SP * O/d)
SW = 8192.0          # PSUM proj-out scale (host divides)
A_EXP = 184.6650390625
B_EXP = 16250.35

F8 = ml_dtypes.float8_e4m3
BF = ml_dtypes.bfloat16

KP = (0, 64, 0)      # K/Q base partition per local head
OP = (64, 0, 64)     # one-hot / rel base partition

_prog_cache = {}


def _pack6(w):
    # (768, M) -> [128, 6*M]: contraction chunk c of 128 at cols [c*M:(c+1)*M]
    m = w.shape[1]
    return np.ascontiguousarray(
        w.reshape(6, 128, m).transpose(1, 0, 2).reshape(128, 6 * m))


def _build_program(taps=False):
    import concourse.bacc as bacc
    import concourse.mybir as mybir
    import concourse.tile as tile
    from contextlib import ExitStack

    f32 = mybir.dt.float32
    bf16 = mybir.dt.bfloat16
    i16 = mybir.dt.int16
    f8 = mybir.dt.float8e4
    AF = mybir.ActivationFunctionType
    ADD = mybir.AluOpType.add
    SUB = mybir.AluOpType.subtract
    MUL = mybir.AluOpType.mult
    DR = mybir.MatmulPerfMode.DoubleRow

    nc = bacc.Bacc("TRN2", target_bir_lowering=False, debug=False)

    XTB = nc.dram_tensor("xtb", [128, 6 * N], bf16, kind="ExternalInput")
    XTQ = nc.dram_tensor("xtq", [128, 6 * QL], bf16, kind="ExternalInput")
    WKA = nc.dram_tensor("wka", [128, 6 * 128], bf16, kind="ExternalInput")
    WKC = nc.dram_tensor("wkc", [128, 6 * 64], bf16, kind="ExternalInput")
    WQA = nc.dram_tensor("wqa", [128, 6 * 128], bf16, kind="ExternalInput")
    WQC = nc.dram_tensor("wqc", [128, 6 * 64], bf16, kind="ExternalInput")
    WVB = nc.dram_tensor("wvb", [128, 6 * 192], bf16, kind="ExternalInput")
    BKA = nc.dram_tensor("bka", [128, 1], f32, kind="ExternalInput")
    BKC = nc.dram_tensor("bkc", [64, 1], f32, kind="ExternalInput")
    BQA = nc.dram_tensor("bqa", [128, 1], f32, kind="ExternalInput")
    BQC = nc.dram_tensor("bqc", [64, 1], f32, kind="ExternalInput")
    BVR = nc.dram_tensor("bvr", [1, 192], bf16, kind="ExternalInput")
    RHT = nc.dram_tensor("rht", [128, 32 * 64], bf16, kind="ExternalInput")
    RWT = nc.dram_tensor("rwt", [128, 64 * 64], bf16, kind="ExternalInput")
    IDK = nc.dram_tensor("idk", [64, 32 * 256], f8, kind="ExternalInput")
    PWH = nc.dram_tensor("pwh", [96, 6 * 256], f8, kind="ExternalInput")
    PWL = nc.dram_tensor("pwl", [96, 6 * 256], f8, kind="ExternalInput")
    OUT = nc.dram_tensor("out", [C, QL], bf16, kind="ExternalOutput")

    if taps:
        TKA = nc.dram_tensor("t_ka", [128, HPG * 8192], f8, kind="ExternalOutput")
        TQA = nc.dram_tensor("t_qa", [128, HPG * 4096], f8, kind="ExternalOutput")
        TVN = nc.dram_tensor("t_vn", [128, 32 * 240], f8, kind="ExternalOutput")
        TS0 = nc.dram_tensor("t_s0", [128, 1024], f32, kind="ExternalOutput")
        TE8 = nc.dram_tensor("t_e8", [128, 2048], f8, kind="ExternalOutput")
        TPRJ = nc.dram_tensor("t_prj", [96, 4096], f8, kind="ExternalOutput")

    with tile.TileContext(nc) as tc, ExitStack() as es:
        const = es.enter_context(tc.tile_pool(name="const", bufs=1))
        big = es.enter_context(tc.tile_pool(name="big", bufs=1))
        xp = es.enter_context(tc.tile_pool(name="xp", bufs=2))
        p1 = es.enter_context(tc.tile_pool(name="p1", bufs=2, space="PSUM"))
        scp = es.enter_context(tc.tile_pool(name="sc", bufs=2, space="PSUM"))
        ovp = es.enter_context(tc.tile_pool(name="ov", bufs=1, space="PSUM"))
        e8p = es.enter_context(tc.tile_pool(name="e8p", bufs=2))
        e16p = es.enter_context(tc.tile_pool(name="e16p", bufs=3))
        nrm = es.enter_context(tc.tile_pool(name="nrm", bufs=2))
        stg = es.enter_context(tc.tile_pool(name="stg", bufs=3))

        # ---- persistent tiles ----
        wka_t = const.tile([128, 6 * 128], bf16, tag="wka", name="wka")
        wkc_t = const.tile([128, 6 * 64], bf16, tag="wkc", name="wkc")
        wqa_t = const.tile([128, 6 * 128], bf16, tag="wqa", name="wqa")
        wqc_t = const.tile([128, 6 * 64], bf16, tag="wqc", name="wqc")
        wvb_t = const.tile([128, 6 * 192], bf16, tag="wvb", name="wvb")
        bka_t = const.tile([128, 1], f32, tag="bka", name="bka")
        bkc_t = const.tile([64, 1], f32, tag="bkc", name="bkc")
        bqa_t = const.tile([128, 1], f32, tag="bqa", name="bqa")
        bqc_t = const.tile([64, 1], f32, tag="bqc", name="bqc")
        bvr_t = const.tile([1, 192], bf16, tag="bvr", name="bvr")
        rht_t = const.tile([128, 32 * 64], bf16, tag="rht", name="rht")
        rwt_t = const.tile([128, 64 * 64], bf16, tag="rwt", name="rwt")
        pwh_t = const.tile([96, 6 * 256], f8, tag="pwh", name="pwh")
        pwl_t = const.tile([96, 6 * 256], f8, tag="pwl", name="pwl")
        ones1b = const.tile([1, 128], bf16, tag="ones1b", name="ones1b")

        for t_, d_ in [(wka_t, WKA), (wkc_t, WKC), (wqa_t, WQA), (wqc_t, WQC),
                       (wvb_t, WVB), (bka_t, BKA), (bkc_t, BKC), (bqa_t, BQA),
                       (bqc_t, BQC), (bvr_t, BVR), (rht_t, RHT), (rwt_t, RWT),
                       (pwh_t, PWH), (pwl_t, PWL)]:
            nc.sync.dma_start(t_[:], d_.ap())
        nc.vector.memset(ones1b[:], 1.0)

        KA = big.tile([128, HPG * 8192], f8, tag="ka", name="ka")
        QA = big.tile([128, HPG * 4096], f8, tag="qa", name="qa")
        VN8 = big.tile([128, 32 * 240], f8, tag="vn", name="vn")
        PRJ8 = big.tile([96, 2 * QL], f8, tag="prj", name="prj")

        # one-hot blocks into KA (shared pattern, per-head partition placement)
        for h in range(HPG):
            nc.sync.dma_start(KA[OP[h]:OP[h] + 64, 8192 * h:8192 * h + 8192],
                              IDK.ap())
        vn3 = VN8[:].rearrange("p (kc x) -> p kc x", x=240)
        for h in range(HPG):
            nc.vector.memset(vn3[:, :, 64 + 80 * h], 1.0)

        # ---- Q projection + fp8 hi/lo stores ----
        def emit_qgen(i):
            xt = xp.tile([128, 6 * 512], bf16, tag="xt", name="xq")
            nc.sync.dma_start(
                xt[:].rearrange("p (cb t) -> p cb t", cb=6),
                XTQ.ap().rearrange("p (cb t) -> p cb t", cb=6)[:, :, 512 * i:512 * i + 512])
            psq = p1.tile([128, 512], f32, tag="p1", name="psq")
            for c in range(6):
                nc.tensor.matmul(psq[:], wqa_t[:, 128 * c:128 * c + 128],
                                 xt[:, 512 * c:512 * c + 512],
                                 start=(c == 0), stop=(c == 5))
            psq2 = p1.tile([64, 512], f32, tag="p1", name="psq2")
            for c in range(6):
                nc.tensor.matmul(psq2[:], wqc_t[:, 64 * c:64 * c + 64],
                                 xt[:, 512 * c:512 * c + 512],
                                 start=(c == 0), stop=(c == 5))
            qc, qo = i // 2, 512 * (i % 2)
            for h in range(HPG):
                if h == 0:
                    src, bias = psq[0:64, :], bqa_t[0:64, :]
                elif h == 1:
                    src, bias = psq[64:128, :], bqa_t[64:128, :]
                else:
                    src, bias = psq2[:], bqc_t[:]
                b = KP[h]
                col = 4096 * h + 2048 * qc + qo
                hi = QA[b:b + 64, col:col + 512]
                nc.vector.tensor_scalar(hi, src, bias, None, ADD)
                lo = QA[b:b + 64, col + 1024:col + 1024 + 512]
                nc.vector.scalar_tensor_tensor(lo, src, bias, hi, ADD, SUB)

        # ---- K + V generation for token chunk t (global) ----
        def emit_kv(t):
            xt = xp.tile([128, 6 * 512], bf16, tag="xt", name="xk")
            nc.sync.dma_start(
                xt[:].rearrange("p (cb t) -> p cb t", cb=6),
                XTB.ap().rearrange("p (cb t) -> p cb t", cb=6)[:, :, 512 * t:512 * t + 512])
            psk = p1.tile([128, 512], f32, tag="p1", name="psk")
            for c in range(6):
                nc.tensor.matmul(psk[:], wka_t[:, 128 * c:128 * c + 128],
                                 xt[:, 512 * c:512 * c + 512],
                                 start=(c == 0), stop=(c == 5))
            psk2 = p1.tile([64, 512], f32, tag="p1", name="psk2")
            for c in range(6):
                nc.tensor.matmul(psk2[:], wkc_t[:, 64 * c:64 * c + 64],
                                 xt[:, 512 * c:512 * c + 512],
                                 start=(c == 0), stop=(c == 5))
            for h in range(HPG):
                if h == 0:
                    src, bias = psk[0:64, :], bka_t[0:64, :]
                elif h == 1:
                    src, bias = psk[64:128, :], bka_t[64:128, :]
                else:
                    src, bias = psk2[:], bkc_t[:]
                b = KP[h]
                srcv = src.rearrange("p (kc m) -> p kc m", m=128)
                base = 8192 * h + 1024 * t
                kslab = KA[b:b + 64, base:base + 1024].rearrange(
                    "p (kc two m) -> p kc two m", two=2, m=128)
                for s in range(2):
                    nc.scalar.activation(kslab[:, :, s, :], srcv, AF.Identity,
                                         bias=bias, scale=1.0)
            for sub in range(4):
                pv = p1.tile([128, 192], f32, tag="p1", name="pv")
                for c in range(6):
                    nc.tensor.matmul(pv[:], xt[:, 512 * c + 128 * sub:512 * c + 128 * sub + 128],
                                     wvb_t[:, 192 * c:192 * c + 192],
                                     start=(c == 0), stop=False)
                nc.tensor.matmul(pv[:], ones1b[:], bvr_t[:], start=False, stop=True)
                kc = 4 * t + sub
                vdst = vn3[:, kc, :].rearrange("p (h x) -> p h x", x=80)[:, :, 0:64]
                nc.scalar.activation(vdst, pv[:].rearrange("p (h x) -> p h x", x=64),
                                     AF.Copy, scale=SV)

        # ---- rel-pos generation for head h ----
        def emit_relgen(h):
            b, ob = KP[h], OP[h]
            qh = QA[b:b + 64, 4096 * h:4096 * h + 4096].rearrange(
                "p (qc s q) -> p qc s q", qc=2, s=2)[:, :, 0, :]   # [64, 2, 1024] Q_hi
            for gi in range(4):
                ps = p1.tile([64, 512], f32, tag="p1", name="prh")
                for ii in range(8):
                    i = 8 * gi + ii     # query-row block (64 queries)
                    qcv = qh[:, i // 16, :].rearrange("p (i q) -> p i q", q=64)[:, i % 16, :]
                    nc.tensor.matmul(ps[:, 64 * ii:64 * ii + 64],
                                     rht_t[b:b + 64, 64 * i:64 * i + 64],
                                     qcv, start=True, stop=True)
                col = 4096 * h + 2048 * (gi // 2) + 512 * (gi % 2)
                nc.scalar.activation(QA[ob:ob + 64, col:col + 512], ps[:],
                                     AF.Copy, scale=1.0)
            qw = qh.rearrange("p qc (i w) -> p qc i w", w=64)   # [64, 2, 16, 64]
            for gi in range(4):
                ps = p1.tile([64, 512], f32, tag="p1", name="prw")
                for wi in range(16):
                    w = 16 * gi + wi
                    nc.tensor.matmul(ps[:, 32 * wi:32 * wi + 32],
                                     rwt_t[b:b + 64, 64 * w:64 * w + 64],
                                     qw[:, :, :, w], start=True, stop=True)
                dst = QA[ob:ob + 64, 4096 * h:4096 * h + 4096].rearrange(
                    "p (qc s q) -> p qc s q", qc=2, s=2)[:, :, 1, :].rearrange(
                    "p qc (i w) -> p qc i w", w=64)[:, :, :, 16 * gi:16 * gi + 16]
                nc.scalar.activation(
                    dst, ps[:].rearrange("p (w qc i) -> p qc i w", w=16, qc=2),
                    AF.Copy, scale=1.0)

        # ---- attention stream ----
        O_PS = {}

        def emit_pair(h, qc, pi):
            """kc pair (2*pi, 2*pi+1); even pi -> ACT/fp8-DR, odd -> DVE/schr."""
            if pi == 0:
                O_PS[(h, qc)] = ovp.tile([65, 1024], f32, tag="ov", name="ov")
            O_ps = O_PS[(h, qc)]
            ka_sl = [KA[:, 8192 * h + 256 * (2 * pi + x):8192 * h + 256 * (2 * pi + x) + 256]
                     .rearrange("p (two m) -> p two m", two=2) for x in range(2)]
            qa_sl = QA[:, 4096 * h + 2048 * qc:4096 * h + 2048 * qc + 2048].rearrange(
                "p (two n) -> p two n", two=2)
            if pi % 2 == 0:
                S0 = scp.tile([128, 1024], f32, tag="sc", name="s0")
                nc.tensor.matmul(S0[:], ka_sl[0], qa_sl, start=True, stop=True,
                                 perf_mode=DR)
                S1 = scp.tile([128, 1024], f32, tag="sc", name="s1")
                nc.tensor.matmul(S1[:], ka_sl[1], qa_sl, start=True, stop=True,
                                 perf_mode=DR)
                e8 = e8p.tile([128, 2048], f8, tag="e8", name="e8")
                nc.scalar.activation(e8[:, 0:1024], S0[:], AF.Exp, scale=1.0 / SEXP)
                nc.scalar.activation(e8[:, 1024:2048], S1[:], AF.Exp, scale=1.0 / SEXP)
                if taps and h == 0 and qc == 0 and pi == 0:
                    sstage = nrm.tile([128, 1024], f32, tag="bsb", name="sstage")
                    nc.vector.tensor_copy(sstage[:], S0[:])
                    nc.sync.dma_start(TS0.ap(), sstage[:])
                    nc.sync.dma_start(TE8.ap(), e8[:])
                vdr = vn3[:, 2 * pi:2 * pi + 2, 80 * h:80 * h + 65]
                nc.tensor.matmul(O_ps[:], vdr,
                                 e8[:].rearrange("p (two n) -> p two n", two=2),
                                 start=(pi == 0), stop=(pi == 15), perf_mode=DR)
            else:
                for x in range(2):
                    Sx = scp.tile([128, 1024], f32, tag="sc", name="sx")
                    nc.tensor.matmul(Sx[:], ka_sl[x], qa_sl, start=True, stop=True,
                                     perf_mode=DR)
                    e16 = e16p.tile([128, 1024], i16, tag="e16", name="e16")
                    nc.vector.tensor_scalar(e16[:], Sx[:], A_EXP / SEXP, B_EXP,
                                            MUL, ADD)
                    nc.tensor.matmul(O_ps[:], vn3[:, 2 * pi + x, 80 * h:80 * h + 65],
                                     e16[:].bitcast(bf16),
                                     start=False, stop=(pi == 15 and x == 1))

        # ---- normalization: PRJ8 = (SP/SV) * O/d, fp8 ----
        RECTS = [  # (head, prj_part0, prj_slot, o_row0, nrows)
            (0, 0, 0, 0, 64),
            (1, 64, 0, 0, 32), (1, 0, 1, 32, 32),
            (2, 32, 1, 0, 64),
        ]

        def emit_norm(h, qc):
            O_ps = O_PS.pop((h, qc))
            rec = nrm.tile([1, 1024], f32, tag="rec", name="rec")
            nc.vector.reciprocal_approx_fast(rec[:], O_ps[64:65, :])
            bsb = nrm.tile([128, 1024], f32, tag="bsb", name="bsb")
            nc.gpsimd.partition_broadcast(bsb[:], rec[:])
            prjv = PRJ8[:].rearrange("p (s q) -> p s q", s=2)
            for (hh, p0, sl, o0, nr) in RECTS:
                if hh != h:
                    continue
                dst = prjv[p0:p0 + nr, sl, 1024 * qc:1024 * qc + 1024]
                nc.vector.scalar_tensor_tensor(
                    dst, O_ps[o0:o0 + nr, :], SP / SV, bsb[o0:o0 + nr, :],
                    MUL, MUL)

        # ---- output projection (fp8-DR hi/lo), DMA from PSUM ----
        def emit_proj(qc, m, wh):
            pp = p1.tile([128, 512], f32, tag="p1", name="pp")
            mv = PRJ8[:].rearrange("p (s q) -> p s q", s=2)[
                :, :, 1024 * qc + 512 * wh:1024 * qc + 512 * wh + 512]
            nc.tensor.matmul(pp[:], pwh_t[:, 256 * m:256 * m + 256]
                             .rearrange("p (two c) -> p two c", two=2),
                             mv, start=True, stop=False, perf_mode=DR)
            nc.tensor.matmul(pp[:], pwl_t[:, 256 * m:256 * m + 256]
                             .rearrange("p (two c) -> p two c", two=2),
                             mv, start=False, stop=True, perf_mode=DR)
            ost = stg.tile([128, 512], bf16, tag="ost", name="ost")
            if (2 * m + wh) % 2 == 0:
                nc.vector.tensor_copy(ost[:], pp[:])
            else:
                nc.scalar.activation(ost[:], pp[:], AF.Copy)
            nc.sync.dma_start(
                OUT.ap()[128 * m:128 * m + 128,
                         1024 * qc + 512 * wh:1024 * qc + 512 * wh + 512], ost[:])

        # ================= schedule =================
        for i in range(4):
            emit_qgen(i)
        emit_relgen(0)
        # (0,0) stream chases K/V generation
        emit_kv(0)
        emit_kv(1)
        for pi in range(16):
            if pi < 6 and pi % 2 == 0:
                emit_kv(2 + pi // 2)        # t=2..4
            if pi in (7, 9, 11):
                emit_kv(5 + (pi - 7) // 2)  # t=5..7
            emit_pair(0, 0, pi)
        emit_norm(0, 0)
        emit_relgen(1)
        for pi in range(16):
            emit_pair(0, 1, pi)
        emit_norm(0, 1)
        for pi in range(16):
            emit_pair(1, 0, pi)
        emit_norm(1, 0)
        emit_relgen(2)
        for pi in range(16):
            emit_pair(1, 1, pi)
        emit_norm(1, 1)
        for pi in range(16):
            emit_pair(2, 0, pi)
        emit_norm(2, 0)
        for pi in range(16):
            emit_pair(2, 1, pi)
            # hide qc0 output projection under the last attention segment
            if pi >= 4 and pi % 2 == 0:
                m = (pi - 4) // 2
                emit_proj(0, m, 0)
                emit_proj(0, m, 1)
        emit_norm(2, 1)

        if taps:
            nc.sync.dma_start(TKA.ap(), KA[:])
            nc.sync.dma_start(TQA.ap(), QA[:])
            nc.sync.dma_start(TVN.ap(), VN8[:])
            nc.sync.dma_start(TPRJ.ap(), PRJ8[:])

        for m in range(6):
            emit_proj(1, m, 0)
            emit_proj(1, m, 1)

    nc.compile()
    return nc


def _host_inputs(x, qkv_w, qkv_b, proj_w, rel_pos_h, rel_pos_w):
    xm = np.ascontiguousarray(x.reshape(N, C)).astype(np.float32)
    xT = xm.T  # (C, N)
    xtb = np.ascontiguousarray(
        xT.reshape(6, 128, N).transpose(1, 0, 2).reshape(128, 6 * N)).astype(BF)

    idx = np.arange(64)[:, None] - np.arange(64)[None, :] + 63
    rh_g = rel_pos_h[idx] * (SREL / SQ)   # (hrow, kh, c)
    rw_g = rel_pos_w[idx] * (SREL / SQ)   # (w, kw, c)
    rwt1 = np.ascontiguousarray(
        rw_g.transpose(2, 0, 1).reshape(64, 64 * 64)).astype(BF)
    rwt = np.concatenate([rwt1, rwt1], axis=0)  # dup rows -> [128, 4096]

    # one-hot block [64, 32*2*128]
    kcs = np.arange(32)
    m = np.arange(128)
    jj = np.arange(64)
    idkh = (jj[:, None, None] == (2 * kcs[None, :, None] + m[None, None, :] // 64))
    idkw = (jj[:, None, None] == (m[None, None, :] % 64))[:, [0] * 32, :] \
        if False else np.broadcast_to(
            (jj[:, None] == (m[None, :] % 64))[:, None, :], (64, 32, 128))
    idk = np.zeros((64, 32, 2, 128), dtype=np.float32)
    idk[:, :, 0, :] = idkh
    idk[:, :, 1, :] = idkw
    idk = np.ascontiguousarray(idk.reshape(64, 32 * 256)).astype(F8)

    in_maps = []
    for core in range(8):
        g, j = core // QB, core % QB
        cs = slice(192 * g, 192 * g + 192)
        wq = qkv_w[:, 0 * C:1 * C][:, cs] * SQ
        wk = qkv_w[:, 1 * C:2 * C][:, cs] * (SCALE * SK)
        wv = qkv_w[:, 2 * C:3 * C][:, cs]
        bq = qkv_b[0 * C:1 * C][cs] * SQ
        bk = qkv_b[1 * C:2 * C][cs] * (SCALE * SK)
        bv = qkv_b[2 * C:3 * C][cs]

        h0 = 32 * j
        rht1 = np.ascontiguousarray(
            rh_g[h0:h0 + 32].transpose(2, 0, 1).reshape(64, 32 * 64)).astype(BF)
        rht = np.concatenate([rht1, rht1], axis=0)

        pw = proj_w[cs] * (SW / SP)        # (192, 768)
        pw96 = pw.reshape(2, 96, 768).transpose(1, 0, 2)   # [96, s, 768]
        pwp = np.ascontiguousarray(
            pw96.reshape(96, 2, 6, 128).transpose(0, 2, 1, 3).reshape(96, 6 * 256))
        pwh = pwp.astype(F8)
        pwl = (pwp - pwh.astype(np.float32)).astype(F8)

        xtq = np.ascontiguousarray(
            xT[:, QL * j:QL * j + QL].reshape(6, 128, QL)
            .transpose(1, 0, 2).reshape(128, 6 * QL)).astype(BF)

        in_maps.append({
            "xtb": xtb,
            "xtq": xtq,
            "wka": _pack6(wk[:, 0:128]).astype(BF),
            "wkc": _pack6(wk[:, 128:192]).astype(BF),
            "wqa": _pack6(wq[:, 0:128]).astype(BF),
            "wqc": _pack6(wq[:, 128:192]).astype(BF),
            "wvb": _pack6(wv).astype(BF),
            "bka": np.ascontiguousarray(bk[0:128, None].astype(np.float32)),
            "bkc": np.ascontiguousarray(bk[128:192, None].astype(np.float32)),
            "bqa": np.ascontiguousarray(bq[0:128, None].astype(np.float32)),
            "bqc": np.ascontiguousarray(bq[128:192, None].astype(np.float32)),
            "bvr": np.ascontiguousarray(bv[None, :]).astype(BF),
            "rht": rht,
            "rwt": rwt,
            "idk": idk,
            "pwh": pwh,
            "pwl": pwl,
        })
    return in_maps


def kernel(x, qkv_w, qkv_b, proj_w, proj_b, rel_pos_h, rel_pos_w):
    from concourse.bass_utils import run_bass_kernel_spmd

    x = np.asarray(x, dtype=np.float32)
    qkv_w = np.asarray(qkv_w, dtype=np.float32)
    qkv_b = np.asarray(qkv_b, dtype=np.float32)
    proj_w = np.asarray(proj_w, dtype=np.float32)
    proj_b = np.asarray(proj_b, dtype=np.float32)
    rel_pos_h = np.asarray(rel_pos_h, dtype=np.float32)
    rel_pos_w = np.asarray(rel_pos_w, dtype=np.float32)

    if "nc" not in _prog_cache:
        _prog_cache["nc"] = _build_program()
    nc = _prog_cache["nc"]

    in_maps = _host_inputs(x, qkv_w, qkv_b, proj_w, rel_pos_h, rel_pos_w)
    res = run_bass_kernel_spmd(nc, in_maps, core_ids=list(range(8)))

    out = np.zeros((N, C), dtype=np.float32)
    for core in range(8):
        g, j = core // QB, core % QB
        out[QL * j:QL * j + QL, :] += res.results[core]["out"].astype(np.float32).T / SW
    out += proj_b[None, :]
    return out.reshape(1, H, W, C).astype(np.float32)


# revision 8
# speedup vs baseline: 1.2114x; 1.0036x over previous
"""Trainium2 Bass kernel for ViT-style attention with decomposed relative
position bias (fp8 DoubleRow edition).

Problem: x(1,64,64,768) -> qkv proj -> 12-head attention with rel_pos_h/w
decomposed bias -> softmax -> out proj.  N=4096 tokens, hd=64.

Sharding: 8 cores = 4 head-groups (3 heads) x 2 query-blocks (2048 queries).

Per-core design (all matmul-heavy paths in fp8 DoubleRow at 0.5 cyc/row):
- Scores: ONE fp8-DR matmul per 128-key chunk folds everything:
    stationary slots  p0-63:(K,K)dup   p64-127:(IDKH,IDKW) one-hots
    moving   slots  p0-63:(Qhi,Qlo)  p64-127:(RH,RW)
  => S = K*(Qhi+Qlo) + rel_h + rel_w, K 8-bit/Q 12-bit, pre-scaled by
  SK*SQ=64 (fp8 range), un-scaled inside exp.
- exp split across engines: even kc-pairs on ACT (true exp -> fp8 E, AV is a
  fp8-DR pair matmul), odd pairs on DVE via Schraudolph bit-trick exp
  (int16 = S*a+b, bitcast bf16; AV is bf16 matmul on fp8 V stationary).
- Softmax denominators from a ones-column in the V stationary; 1/d via
  reciprocal_approx_fast (DVE), broadcast on GPSIMD, fused normalize-mul
  into fp8 PRJ tile (DVE).
- K/Q/V generation in bf16 (exact-ish); K/V/rel converts on ACT
  (Identity/Copy activations share the exp table set -> no table reloads);
  output projection as fp8-DR (hi/lo weights), DMA'd straight from PSUM.
- Head-alternating partition layout (h0:p0-63, h1:p64-127, h2:p0-63 for
  K/Q; one-hots/rel on the complement) so gen-PSUM rows map 1:1.
"""

import numpy as np
import ml_dtypes

NH, HD, C, H, W = 12, 64, 768, 64, 64
N = H * W            # 4096
G, QB = 4, 2         # head groups x query blocks = 8 cores
HPG = NH // G        # 3 heads per group
QL = N // QB         # 2048 queries per block
SCALE = HD ** -0.5

SK, SQ, SREL, SV = 16.0, 4.0, 64.0, 16.0
SEXP = SK * SQ       # scores arrive in PSUM scaled by 64
SP = 256.0           # PRJ tile scale (PRJ = SP * O/d)
SW = 8192.0          # PSUM proj-out scale (host divides)
A_EXP = 184.6650390625
B_EXP = 16250.35

F8 = ml_dtypes.float8_e4m3
BF = ml_dtypes.bfloat16

KP = (0, 64, 0)      # K/Q base partition per local head
OP = (64, 0, 64)     # one-hot / rel base partition

_prog_cache = {}


def _pack6(w):
    # (768, M) -> [128, 6*M]: contraction chunk c of 128 at cols [c*M:(c+1)*M]
    m = w.shape[1]
    return np.ascontiguousarray(
        w.reshape(6, 128, m).transpose(1, 0, 2).reshape(128, 6 * m))


def _build_program(taps=False):
    import concourse.bacc as bacc
    import concourse.mybir as mybir
    import concourse.tile as tile
    from contextlib import ExitStack

    f32 = mybir.dt.float32
    bf16 = mybir.dt.bfloat16
    i16 = mybir.dt.int16
    f8 = mybir.dt.float8e4
    AF = mybir.ActivationFunctionType
    ADD = mybir.AluOpType.add
    SUB = mybir.AluOpType.subtract
    MUL = mybir.AluOpType.mult
    DR = mybir.MatmulPerfMode.DoubleRow

    nc = bacc.Bacc("TRN2", target_bir_lowering=False, debug=False)

    XTB = nc.dram_tensor("xtb", [128, 6 * N], bf16, kind="ExternalInput")
    XTQ = nc.dram_tensor("xtq", [128, 6 * QL], bf16, kind="ExternalInput")
    WKA = nc.dram_tensor("wka", [128, 6 * 128], bf16, kind="ExternalInput")
    WKC = nc.dram_tensor("wkc", [128, 6 * 64], bf16, kind="ExternalInput")
    WQA = nc.dram_tensor("wqa", [128, 6 * 128], bf16, kind="ExternalInput")
    WQC = nc.dram_tensor("wqc", [128, 6 * 64], bf16, kind="ExternalInput")
    WVB = nc.dram_tensor("wvb", [128, 6 * 192], bf16, kind="ExternalInput")
    BKA = nc.dram_tensor("bka", [128, 1], f32, kind="ExternalInput")
    BKC = nc.dram_tensor("bkc", [64, 1], f32, kind="ExternalInput")
    BQA = nc.dram_tensor("bqa", [128, 1], f32, kind="ExternalInput")
    BQC = nc.dram_tensor("bqc", [64, 1], f32, kind="ExternalInput")
    BVR = nc.dram_tensor("bvr", [1, 192], bf16, kind="ExternalInput")
    RHT = nc.dram_tensor("rht", [128, 32 * 64], bf16, kind="ExternalInput")
    RWT = nc.dram_tensor("rwt", [128, 64 * 64], bf16, kind="ExternalInput")
    IDK = nc.dram_tensor("idk", [64, 32 * 256], f8, kind="ExternalInput")
    PWH = nc.dram_tensor("pwh", [96, 6 * 256], f8, kind="ExternalInput")
    PWL = nc.dram_tensor("pwl", [96, 6 * 256], f8, kind="ExternalInput")
    OUT = nc.dram_tensor("out", [C, QL], bf16, kind="ExternalOutput")

    if taps:
        TKA = nc.dram_tensor("t_ka", [128, HPG * 8192], f8, kind="ExternalOutput")
        TQA = nc.dram_tensor("t_qa", [128, HPG * 4096], f8, kind="ExternalOutput")
        TVN = nc.dram_tensor("t_vn", [128, 32 * 240], f8, kind="ExternalOutput")
        TS0 = nc.dram_tensor("t_s0", [128, 1024], f32, kind="ExternalOutput")
        TE8 = nc.dram_tensor("t_e8", [128, 2048], f8, kind="ExternalOutput")
        TPRJ = nc.dram_tensor("t_prj", [96, 4096], f8, kind="ExternalOutput")

    with tile.TileContext(nc) as tc, ExitStack() as es:
        const = es.enter_context(tc.tile_pool(name="const", bufs=1))
        big = es.enter_context(tc.tile_pool(name="big", bufs=1))
        xp = es.enter_context(tc.tile_pool(name="xp", bufs=2))
        p1 = es.enter_context(tc.tile_pool(name="p1", bufs=2, space="PSUM"))
        scp = es.enter_context(tc.tile_pool(name="sc", bufs=2, space="PSUM"))
        ovp = es.enter_context(tc.tile_pool(name="ov", bufs=1, space="PSUM"))
        e8p = es.enter_context(tc.tile_pool(name="e8p", bufs=2))
        e16p = es.enter_context(tc.tile_pool(name="e16p", bufs=3))
        nrm = es.enter_context(tc.tile_pool(name="nrm", bufs=2))
        stg = es.enter_context(tc.tile_pool(name="stg", bufs=3))

        # ---- persistent tiles ----
        wka_t = const.tile([128, 6 * 128], bf16, tag="wka", name="wka")
        wkc_t = const.tile([128, 6 * 64], bf16, tag="wkc", name="wkc")
        wqa_t = const.tile([128, 6 * 128], bf16, tag="wqa", name="wqa")
        wqc_t = const.tile([128, 6 * 64], bf16, tag="wqc", name="wqc")
        wvb_t = const.tile([128, 6 * 192], bf16, tag="wvb", name="wvb")
        bka_t = const.tile([128, 1], f32, tag="bka", name="bka")
        bkc_t = const.tile([64, 1], f32, tag="bkc", name="bkc")
        bqa_t = const.tile([128, 1], f32, tag="bqa", name="bqa")
        bqc_t = const.tile([64, 1], f32, tag="bqc", name="bqc")
        bvr_t = const.tile([1, 192], bf16, tag="bvr", name="bvr")
        rht_t = const.tile([128, 32 * 64], bf16, tag="rht", name="rht")
        rwt_t = const.tile([128, 64 * 64], bf16, tag="rwt", name="rwt")
        pwh_t = const.tile([96, 6 * 256], f8, tag="pwh", name="pwh")
        pwl_t = const.tile([96, 6 * 256], f8, tag="pwl", name="pwl")
        ones1b = const.tile([1, 128], bf16, tag="ones1b", name="ones1b")

        for t_, d_ in [(wka_t, WKA), (wkc_t, WKC), (wqa_t, WQA), (wqc_t, WQC),
                       (wvb_t, WVB), (bka_t, BKA), (bkc_t, BKC), (bqa_t, BQA),
                       (bqc_t, BQC), (bvr_t, BVR), (rht_t, RHT), (rwt_t, RWT),
                       (pwh_t, PWH), (pwl_t, PWL)]:
            nc.sync.dma_start(t_[:], d_.ap())
        nc.vector.memset(ones1b[:], 1.0)

        KA = big.tile([128, HPG * 8192], f8, tag="ka", name="ka")
        QA = big.tile([128, HPG * 4096], f8, tag="qa", name="qa")
        VN8 = big.tile([128, 32 * 240], f8, tag="vn", name="vn")
        PRJ8 = big.tile([96, 2 * QL], f8, tag="prj", name="prj")

        # one-hot blocks into KA (shared pattern, per-head partition placement)
        for h in range(HPG):
            nc.sync.dma_start(KA[OP[h]:OP[h] + 64, 8192 * h:8192 * h + 8192],
                              IDK.ap())
        vn3 = VN8[:].rearrange("p (kc x) -> p kc x", x=240)
        for h in range(HPG):
            nc.vector.memset(vn3[:, :, 64 + 80 * h], 1.0)

        # ---- Q projection + fp8 hi/lo stores ----
        def emit_qgen(i):
            xt = xp.tile([128, 6 * 512], bf16, tag="xt", name="xq")
            nc.sync.dma_start(
                xt[:].rearrange("p (cb t) -> p cb t", cb=6),
                XTQ.ap().rearrange("p (cb t) -> p cb t", cb=6)[:, :, 512 * i:512 * i + 512])
            psq = p1.tile([128, 512], f32, tag="p1", name="psq")
            for c in range(6):
                nc.tensor.matmul(psq[:], wqa_t[:, 128 * c:128 * c + 128],
                                 xt[:, 512 * c:512 * c + 512],
                                 start=(c == 0), stop=(c == 5))
            psq2 = p1.tile([64, 512], f32, tag="p1", name="psq2")
            for c in range(6):
                nc.tensor.matmul(psq2[:], wqc_t[:, 64 * c:64 * c + 64],
                                 xt[:, 512 * c:512 * c + 512],
                                 start=(c == 0), stop=(c == 5))
            qc, qo = i // 2, 512 * (i % 2)
            for h in range(HPG):
                if h == 0:
                    src, bias = psq[0:64, :], bqa_t[0:64, :]
                elif h == 1:
                    src, bias = psq[64:128, :], bqa_t[64:128, :]
                else:
                    src, bias = psq2[:], bqc_t[:]
                b = KP[h]
                col = 4096 * h + 2048 * qc + qo
                hi = QA[b:b + 64, col:col + 512]
                nc.vector.tensor_scalar(hi, src, bias, None, ADD)
                lo = QA[b:b + 64, col + 1024:col + 1024 + 512]
                nc.vector.scalar_tensor_tensor(lo, src, bias, hi, ADD, SUB)

        # ---- K + V generation for token chunk t (global) ----
        def emit_kv(t):
            xt = xp.tile([128, 6 * 512], bf16, tag="xt", name="xk")
            nc.sync.dma_start(
                xt[:].rearrange("p (cb t) -> p cb t", cb=6),
                XTB.ap().rearrange("p (cb t) -> p cb t", cb=6)[:, :, 512 * t:512 * t + 512])
            psk = p1.tile([128, 512], f32, tag="p1", name="psk")
            for c in range(6):
                nc.tensor.matmul(psk[:], wka_t[:, 128 * c:128 * c + 128],
                                 xt[:, 512 * c:512 * c + 512],
                                 start=(c == 0), stop=(c == 5))
            psk2 = p1.tile([64, 512], f32, tag="p1", name="psk2")
            for c in range(6):
                nc.tensor.matmul(psk2[:], wkc_t[:, 64 * c:64 * c + 64],
                                 xt[:, 512 * c:512 * c + 512],
                                 start=(c == 0), stop=(c == 5))
            for h in range(HPG):
                if h == 0:
                    src, bias = psk[0:64, :], bka_t[0:64, :]
                elif h == 1:
                    src, bias = psk[64:128, :], bka_t[64:128, :]
                else:
                    src, bias = psk2[:], bkc_t[:]
                b = KP[h]
                srcv = src.rearrange("p (kc m) -> p kc m", m=128)
                base = 8192 * h + 1024 * t
                kslab = KA[b:b + 64, base:base + 1024].rearrange(
                    "p (kc two m) -> p kc two m", two=2, m=128)
                for s in range(2):
                    nc.scalar.activation(kslab[:, :, s, :], srcv, AF.Identity,
                                         bias=bias, scale=1.0)
            for sub in range(4):
                pv = p1.tile([128, 192], f32, tag="p1", name="pv")
                for c in range(6):
                    nc.tensor.matmul(pv[:], xt[:, 512 * c + 128 * sub:512 * c + 128 * sub + 128],
                                     wvb_t[:, 192 * c:192 * c + 192],
                                     start=(c == 0), stop=False)
                nc.tensor.matmul(pv[:], ones1b[:], bvr_t[:], start=False, stop=True)
                kc = 4 * t + sub
                vdst = vn3[:, kc, :].rearrange("p (h x) -> p h x", x=80)[:, :, 0:64]
                nc.scalar.activation(vdst, pv[:].rearrange("p (h x) -> p h x", x=64),
                                     AF.Copy, scale=SV)

        # ---- rel-pos generation for head h ----
        def emit_relgen(h):
            b, ob = KP[h], OP[h]
            qh = QA[b:b + 64, 4096 * h:4096 * h + 4096].rearrange(
                "p (qc s q) -> p qc s q", qc=2, s=2)[:, :, 0, :]   # [64, 2, 1024] Q_hi
            for gi in range(4):
                ps = p1.tile([64, 512], f32, tag="p1", name="prh")
                for ii in range(8):
                    i = 8 * gi + ii     # query-row block (64 queries)
                    qcv = qh[:, i // 16, :].rearrange("p (i q) -> p i q", q=64)[:, i % 16, :]
                    nc.tensor.matmul(ps[:, 64 * ii:64 * ii + 64],
                                     rht_t[b:b + 64, 64 * i:64 * i + 64],
                                     qcv, start=True, stop=True)
                col = 4096 * h + 2048 * (gi // 2) + 512 * (gi % 2)
                nc.scalar.activation(QA[ob:ob + 64, col:col + 512], ps[:],
                                     AF.Copy, scale=1.0)
            qw = qh.rearrange("p qc (i w) -> p qc i w", w=64)   # [64, 2, 16, 64]
            for gi in range(4):
                ps = p1.tile([64, 512], f32, tag="p1", name="prw")
                for wi in range(16):
                    w = 16 * gi + wi
                    nc.tensor.matmul(ps[:, 32 * wi:32 * wi + 32],
                                     rwt_t[b:b + 64, 64 * w:64 * w + 64],
                                     qw[:, :, :, w], start=True, stop=True)
                dst = QA[ob:ob + 64, 4096 * h:4096 * h + 4096].rearrange(
                    "p (qc s q) -> p qc s q", qc=2, s=2)[:, :, 1, :].rearrange(
                    "p qc (i w) -> p qc i w", w=64)[:, :, :, 16 * gi:16 * gi + 16]
                nc.scalar.activation(
                    dst, ps[:].rearrange("p (w qc i) -> p qc i w", w=16, qc=2),
                    AF.Copy, scale=1.0)

        # ---- attention stream ----
        O_PS = {}

        def _smat(h, qc, kc):
            S = scp.tile([128, 1024], f32, tag="sc", name="s")
            nc.tensor.matmul(
                S[:],
                KA[:, 8192 * h + 256 * kc:8192 * h + 256 * kc + 256]
                .rearrange("p (two m) -> p two m", two=2),
                QA[:, 4096 * h + 2048 * qc:4096 * h + 2048 * qc + 2048]
                .rearrange("p (two n) -> p two n", two=2),
                start=True, stop=True, perf_mode=DR)
            return S

        def emit_group(h, qc, gi):
            """chunks 4gi..4gi+3: ACT on 4gi & 4gi+2 (fp8 + strided DR-AV pair),
            DVE-schraudolph on 4gi+1 & 4gi+3 (bf16 AV)."""
            if gi == 0:
                O_PS[(h, qc)] = ovp.tile([65, 1024], f32, tag="ov", name="ov")
            O_ps = O_PS[(h, qc)]
            k0 = 4 * gi
            S0 = _smat(h, qc, k0)
            S1 = _smat(h, qc, k0 + 1)
            e8 = e8p.tile([128, 2048], f8, tag="e8", name="e8")
            nc.scalar.activation(e8[:, 0:1024], S0[:], AF.Exp, scale=1.0 / SEXP)
            e16a = e16p.tile([128, 1024], i16, tag="e16", name="e16a")
            nc.vector.tensor_scalar(e16a[:], S1[:], A_EXP / SEXP, B_EXP, MUL, ADD)
            if taps and h == 0 and qc == 0 and gi == 0:
                sstage = nrm.tile([128, 1024], f32, tag="bsb", name="sstage")
                nc.vector.tensor_copy(sstage[:], S0[:])
                nc.sync.dma_start(TS0.ap(), sstage[:])
            S2 = _smat(h, qc, k0 + 2)
            S3 = _smat(h, qc, k0 + 3)
            nc.scalar.activation(e8[:, 1024:2048], S2[:], AF.Exp, scale=1.0 / SEXP)
            e16b = e16p.tile([128, 1024], i16, tag="e16", name="e16b")
            nc.vector.tensor_scalar(e16b[:], S3[:], A_EXP / SEXP, B_EXP, MUL, ADD)
            if taps and h == 0 and qc == 0 and gi == 0:
                nc.sync.dma_start(TE8.ap(), e8[:])
            vdr = vn3[:, k0:k0 + 3:2, 80 * h:80 * h + 65]
            nc.tensor.matmul(O_ps[:], vdr,
                             e8[:].rearrange("p (two n) -> p two n", two=2),
                             start=(gi == 0), stop=False, perf_mode=DR)
            nc.tensor.matmul(O_ps[:], vn3[:, k0 + 1, 80 * h:80 * h + 65],
                             e16a[:].bitcast(bf16), start=False, stop=False)
            nc.tensor.matmul(O_ps[:], vn3[:, k0 + 3, 80 * h:80 * h + 65],
                             e16b[:].bitcast(bf16), start=False, stop=(gi == 7))

        # ---- normalization: PRJ8 = (SP/SV) * O/d, fp8 ----
        RECTS = [  # (head, prj_part0, prj_slot, o_row0, nrows)
            (0, 0, 0, 0, 64),
            (1, 64, 0, 0, 32), (1, 0, 1, 32, 32),
            (2, 32, 1, 0, 64),
        ]

        def emit_norm(h, qc):
            O_ps = O_PS.pop((h, qc))
            rec = nrm.tile([1, 1024], f32, tag="rec", name="rec")
            nc.vector.reciprocal_approx_fast(rec[:], O_ps[64:65, :])
            bsb = nrm.tile([128, 1024], f32, tag="bsb", name="bsb")
            nc.gpsimd.partition_broadcast(bsb[:], rec[:])
            prjv = PRJ8[:].rearrange("p (s q) -> p s q", s=2)
            for (hh, p0, sl, o0, nr) in RECTS:
                if hh != h:
                    continue
                dst = prjv[p0:p0 + nr, sl, 1024 * qc:1024 * qc + 1024]
                nc.vector.scalar_tensor_tensor(
                    dst, O_ps[o0:o0 + nr, :], SP / SV, bsb[o0:o0 + nr, :],
                    MUL, MUL)

        # ---- output projection (fp8-DR hi/lo), DMA from PSUM ----
        def emit_proj(qc, m, wh):
            pp = p1.tile([128, 512], f32, tag="p1", name="pp")
            mv = PRJ8[:].rearrange("p (s q) -> p s q", s=2)[
                :, :, 1024 * qc + 512 * wh:1024 * qc + 512 * wh + 512]
            nc.tensor.matmul(pp[:], pwh_t[:, 256 * m:256 * m + 256]
                             .rearrange("p (two c) -> p two c", two=2),
                             mv, start=True, stop=False, perf_mode=DR)
            nc.tensor.matmul(pp[:], pwl_t[:, 256 * m:256 * m + 256]
                             .rearrange("p (two c) -> p two c", two=2),
                             mv, start=False, stop=True, perf_mode=DR)
            ost = stg.tile([128, 512], bf16, tag="ost", name="ost")
            if (2 * m + wh) % 2 == 0:
                nc.vector.tensor_copy(ost[:], pp[:])
            else:
                nc.scalar.activation(ost[:], pp[:], AF.Copy)
            nc.sync.dma_start(
                OUT.ap()[128 * m:128 * m + 128,
                         1024 * qc + 512 * wh:1024 * qc + 512 * wh + 512], ost[:])

        # ================= schedule =================
        for i in range(4):
            emit_qgen(i)
        emit_relgen(0)
        # (0,0) stream chases K/V generation (group gi consumes t-chunk gi)
        emit_kv(0)
        emit_kv(1)
        for gi in range(8):
            emit_group(0, 0, gi)
            if gi < 6:
                emit_kv(gi + 2)
        emit_norm(0, 0)
        emit_relgen(1)
        for gi in range(8):
            emit_group(0, 1, gi)
        emit_norm(0, 1)
        for gi in range(8):
            emit_group(1, 0, gi)
        emit_norm(1, 0)
        emit_relgen(2)
        for gi in range(8):
            emit_group(1, 1, gi)
        emit_norm(1, 1)
        for gi in range(8):
            emit_group(2, 0, gi)
        emit_norm(2, 0)
        for gi in range(8):
            emit_group(2, 1, gi)
            # hide qc0 output projection under the last attention segment
            if 1 <= gi <= 6:
                m = gi - 1
                emit_proj(0, m, 0)
                emit_proj(0, m, 1)
        emit_norm(2, 1)

        if taps:
            nc.sync.dma_start(TKA.ap(), KA[:])
            nc.sync.dma_start(TQA.ap(), QA[:])
            nc.sync.dma_start(TVN.ap(), VN8[:])
            nc.sync.dma_start(TPRJ.ap(), PRJ8[:])

        for m in range(6):
            emit_proj(1, m, 0)
            emit_proj(1, m, 1)

    nc.compile()
    return nc


def _host_inputs(x, qkv_w, qkv_b, proj_w, rel_pos_h, rel_pos_w):
    xm = np.ascontiguousarray(x.reshape(N, C)).astype(np.float32)
    xT = xm.T  # (C, N)
    xtb = np.ascontiguousarray(
        xT.reshape(6, 128, N).transpose(1, 0, 2).reshape(128, 6 * N)).astype(BF)

    idx = np.arange(64)[:, None] - np.arange(64)[None, :] + 63
    rh_g = rel_pos_h[idx] * (SREL / SQ)   # (hrow, kh, c)
    rw_g = rel_pos_w[idx] * (SREL / SQ)   # (w, kw, c)
    rwt1 = np.ascontiguousarray(
        rw_g.transpose(2, 0, 1).reshape(64, 64 * 64)).astype(BF)
    rwt = np.concatenate([rwt1, rwt1], axis=0)  # dup rows -> [128, 4096]

    # one-hot block [64, 32*2*128]
    kcs = np.arange(32)
    m = np.arange(128)
    jj = np.arange(64)
    idkh = (jj[:, None, None] == (2 * kcs[None, :, None] + m[None, None, :] // 64))
    idkw = (jj[:, None, None] == (m[None, None, :] % 64))[:, [0] * 32, :] \
        if False else np.broadcast_to(
            (jj[:, None] == (m[None, :] % 64))[:, None, :], (64, 32, 128))
    idk = np.zeros((64, 32, 2, 128), dtype=np.float32)
    idk[:, :, 0, :] = idkh
    idk[:, :, 1, :] = idkw
    idk = np.ascontiguousarray(idk.reshape(64, 32 * 256)).astype(F8)

    in_maps = []
    for core in range(8):
        g, j = core // QB, core % QB
        cs = slice(192 * g, 192 * g + 192)
        wq = qkv_w[:, 0 * C:1 * C][:, cs] * SQ
        wk = qkv_w[:, 1 * C:2 * C][:, cs] * (SCALE * SK)
        wv = qkv_w[:, 2 * C:3 * C][:, cs]
        bq = qkv_b[0 * C:1 * C][cs] * SQ
        bk = qkv_b[1 * C:2 * C][cs] * (SCALE * SK)
        bv = qkv_b[2 * C:3 * C][cs]

        h0 = 32 * j
        rht1 = np.ascontiguousarray(
            rh_g[h0:h0 + 32].transpose(2, 0, 1).reshape(64, 32 * 64)).astype(BF)
        rht = np.concatenate([rht1, rht1], axis=0)

        pw = proj_w[cs] * (SW / SP)        # (192, 768)
        pw96 = pw.reshape(2, 96, 768).transpose(1, 0, 2)   # [96, s, 768]
        pwp = np.ascontiguousarray(
            pw96.reshape(96, 2, 6, 128).transpose(0, 2, 1, 3).reshape(96, 6 * 256))
        pwh = pwp.astype(F8)
        pwl = (pwp - pwh.astype(np.float32)).astype(F8)

        xtq = np.ascontiguousarray(
            xT[:, QL * j:QL * j + QL].reshape(6, 128, QL)
            .transpose(1, 0, 2).reshape(128, 6 * QL)).astype(BF)

        in_maps.append({
            "xtb": xtb,
            "xtq": xtq,
            "wka": _pack6(wk[:, 0:128]).astype(BF),
            "wkc": _pack6(wk[:, 128:192]).astype(BF),
            "wqa": _pack6(wq[:, 0:128]).astype(BF),
            "wqc": _pack6(wq[:, 128:192]).astype(BF),
            "wvb": _pack6(wv).astype(BF),
            "bka": np.ascontiguousarray(bk[0:128, None].astype(np.float32)),
            "bkc": np.ascontiguousarray(bk[128:192, None].astype(np.float32)),
            "bqa": np.ascontiguousarray(bq[0:128, None].astype(np.float32)),
            "bqc": np.ascontiguousarray(bq[128:192, None].astype(np.float32)),
            "bvr": np.ascontiguousarray(bv[None, :]).astype(BF),
            "rht": rht,
            "rwt": rwt,
            "idk": idk,
            "pwh": pwh,
            "pwl": pwl,
        })
    return in_maps


def kernel(x, qkv_w, qkv_b, proj_w, proj_b, rel_pos_h, rel_pos_w):
    from concourse.bass_utils import run_bass_kernel_spmd

    x = np.asarray(x, dtype=np.float32)
    qkv_w = np.asarray(qkv_w, dtype=np.float32)
    qkv_b = np.asarray(qkv_b, dtype=np.float32)
    proj_w = np.asarray(proj_w, dtype=np.float32)
    proj_b = np.asarray(proj_b, dtype=np.float32)
    rel_pos_h = np.asarray(rel_pos_h, dtype=np.float32)
    rel_pos_w = np.asarray(rel_pos_w, dtype=np.float32)

    if "nc" not in _prog_cache:
        _prog_cache["nc"] = _build_program()
    nc = _prog_cache["nc"]

    in_maps = _host_inputs(x, qkv_w, qkv_b, proj_w, rel_pos_h, rel_pos_w)
    res = run_bass_kernel_spmd(nc, in_maps, core_ids=list(range(8)))

    out = np.zeros((N, C), dtype=np.float32)
    for core in range(8):
        g, j = core // QB, core % QB
        out[QL * j:QL * j + QL, :] += res.results[core]["out"].astype(np.float32).T / SW
    out += proj_b[None, :]
    return out.reshape(1, H, W, C).astype(np.float32)


# revision 13
# speedup vs baseline: 1.3529x; 1.1168x over previous
"""Trainium2 Bass kernel for ViT-style attention with decomposed relative
position bias (fp8 DoubleRow edition).

Problem: x(1,64,64,768) -> qkv proj -> 12-head attention with rel_pos_h/w
decomposed bias -> softmax -> out proj.  N=4096 tokens, hd=64.

Sharding: 8 cores = 4 head-groups (3 heads) x 2 query-blocks (2048 queries).

Per-core design (all matmul-heavy paths in fp8 DoubleRow at 0.5 cyc/row):
- Scores: ONE fp8-DR matmul per 128-key chunk folds everything:
    stationary slots  p0-63:(K,K)dup   p64-127:(IDKH,IDKW) one-hots
    moving   slots  p0-63:(Qhi,Qlo)  p64-127:(RH,RW)
  => S = K*(Qhi+Qlo) + rel_h + rel_w, K 8-bit/Q 12-bit, pre-scaled by
  SK*SQ=64 (fp8 range), un-scaled inside exp.
- exp split across engines: even kc-pairs on ACT (true exp -> fp8 E, AV is a
  fp8-DR pair matmul), odd pairs on DVE via Schraudolph bit-trick exp
  (int16 = S*a+b, bitcast bf16; AV is bf16 matmul on fp8 V stationary).
- Softmax denominators from a ones-column in the V stationary; 1/d via
  reciprocal_approx_fast (DVE), broadcast on GPSIMD, fused normalize-mul
  into fp8 PRJ tile (DVE).
- K/Q/V generation in bf16 (exact-ish); K/V/rel converts on ACT
  (Identity/Copy activations share the exp table set -> no table reloads);
  output projection as fp8-DR (hi/lo weights), DMA'd straight from PSUM.
- Head-alternating partition layout (h0:p0-63, h1:p64-127, h2:p0-63 for
  K/Q; one-hots/rel on the complement) so gen-PSUM rows map 1:1.
"""

import numpy as np
import ml_dtypes

NH, HD, C, H, W = 12, 64, 768, 64, 64
N = H * W            # 4096
G, QB = 4, 2         # head groups x query blocks = 8 cores
HPG = NH // G        # 3 heads per group
QL = N // QB         # 2048 queries per block
SCALE = HD ** -0.5

SK, SQ, SREL, SV = 16.0, 4.0, 64.0, 16.0
SEXP = SK * SQ       # scores arrive in PSUM scaled by 64
SP = 256.0           # PRJ tile scale (PRJ = SP * O/d)
SW = 8192.0          # PSUM proj-out scale (host divides)
A_EXP = 184.6650390625
B_EXP = 16250.35

F8 = ml_dtypes.float8_e4m3
BF = ml_dtypes.bfloat16

KP = (0, 64, 0)      # K/Q base partition per local head
OP = (64, 0, 64)     # one-hot / rel base partition

_prog_cache = {}


def _pack6(w):
    # (768, M) -> [128, 6*M]: contraction chunk c of 128 at cols [c*M:(c+1)*M]
    m = w.shape[1]
    return np.ascontiguousarray(
        w.reshape(6, 128, m).transpose(1, 0, 2).reshape(128, 6 * m))


def _build_program(taps=False):
    import concourse.bacc as bacc
    import concourse.mybir as mybir
    import concourse.tile as tile
    from contextlib import ExitStack

    f32 = mybir.dt.float32
    bf16 = mybir.dt.bfloat16
    i16 = mybir.dt.int16
    f8 = mybir.dt.float8e4
    AF = mybir.ActivationFunctionType
    ADD = mybir.AluOpType.add
    SUB = mybir.AluOpType.subtract
    MUL = mybir.AluOpType.mult
    DR = mybir.MatmulPerfMode.DoubleRow

    nc = bacc.Bacc("TRN2", target_bir_lowering=False, debug=False)

    XTB = nc.dram_tensor("xtb", [128, 6 * N], bf16, kind="ExternalInput")
    XTQ = nc.dram_tensor("xtq", [128, 6 * QL], bf16, kind="ExternalInput")
    WKA = nc.dram_tensor("wka", [128, 6 * 128], bf16, kind="ExternalInput")
    WKC = nc.dram_tensor("wkc", [128, 6 * 64], bf16, kind="ExternalInput")
    WQA = nc.dram_tensor("wqa", [128, 6 * 128], bf16, kind="ExternalInput")
    WQC = nc.dram_tensor("wqc", [128, 6 * 64], bf16, kind="ExternalInput")
    WVB = nc.dram_tensor("wvb", [128, 6 * 192], bf16, kind="ExternalInput")
    BKA = nc.dram_tensor("bka", [128, 1], f32, kind="ExternalInput")
    BKC = nc.dram_tensor("bkc", [64, 1], f32, kind="ExternalInput")
    BQA = nc.dram_tensor("bqa", [128, 1], f32, kind="ExternalInput")
    BQC = nc.dram_tensor("bqc", [64, 1], f32, kind="ExternalInput")
    BVR = nc.dram_tensor("bvr", [1, 192], bf16, kind="ExternalInput")
    RHT = nc.dram_tensor("rht", [128, 32 * 64], bf16, kind="ExternalInput")
    RWT = nc.dram_tensor("rwt", [128, 64 * 64], bf16, kind="ExternalInput")
    IDK = nc.dram_tensor("idk", [64, 32 * 256], f8, kind="ExternalInput")
    PWH = nc.dram_tensor("pwh", [96, 6 * 256], f8, kind="ExternalInput")
    PWL = nc.dram_tensor("pwl", [96, 6 * 256], f8, kind="ExternalInput")
    OUT = nc.dram_tensor("out", [C, QL], bf16, kind="ExternalOutput")

    if taps:
        TKA = nc.dram_tensor("t_ka", [128, HPG * 8192], f8, kind="ExternalOutput")
        TQA = nc.dram_tensor("t_qa", [128, HPG * 4096], f8, kind="ExternalOutput")
        TVN = nc.dram_tensor("t_vn", [128, 32 * 240], f8, kind="ExternalOutput")
        TS0 = nc.dram_tensor("t_s0", [128, 1024], f32, kind="ExternalOutput")
        TE8 = nc.dram_tensor("t_e8", [128, 2048], f8, kind="ExternalOutput")
        TPRJ = nc.dram_tensor("t_prj", [96, 4096], f8, kind="ExternalOutput")

    with tile.TileContext(nc) as tc, ExitStack() as es:
        const = es.enter_context(tc.tile_pool(name="const", bufs=1))
        big = es.enter_context(tc.tile_pool(name="big", bufs=1))
        xp = es.enter_context(tc.tile_pool(name="xp", bufs=2))
        scp = es.enter_context(tc.tile_pool(name="sc", bufs=3, space="PSUM"))
        p1 = scp
        ovp = es.enter_context(tc.tile_pool(name="ov", bufs=1, space="PSUM"))
        e8p = es.enter_context(tc.tile_pool(name="e8p", bufs=2))
        e16p = es.enter_context(tc.tile_pool(name="e16p", bufs=4))
        nrm = es.enter_context(tc.tile_pool(name="nrm", bufs=2))
        stg = es.enter_context(tc.tile_pool(name="stg", bufs=3))

        # ---- persistent tiles ----
        wka_t = const.tile([128, 6 * 128], bf16, tag="wka", name="wka")
        wkc_t = const.tile([128, 6 * 64], bf16, tag="wkc", name="wkc")
        wqa_t = const.tile([128, 6 * 128], bf16, tag="wqa", name="wqa")
        wqc_t = const.tile([128, 6 * 64], bf16, tag="wqc", name="wqc")
        wvb_t = const.tile([128, 6 * 192], bf16, tag="wvb", name="wvb")
        bka_t = const.tile([128, 1], f32, tag="bka", name="bka")
        bkc_t = const.tile([64, 1], f32, tag="bkc", name="bkc")
        bqa_t = const.tile([128, 1], f32, tag="bqa", name="bqa")
        bqc_t = const.tile([64, 1], f32, tag="bqc", name="bqc")
        bvr_t = const.tile([1, 192], bf16, tag="bvr", name="bvr")
        rht_t = const.tile([128, 32 * 64], bf16, tag="rht", name="rht")
        rwt_t = const.tile([128, 64 * 64], bf16, tag="rwt", name="rwt")
        pwh_t = const.tile([96, 6 * 256], f8, tag="pwh", name="pwh")
        pwl_t = const.tile([96, 6 * 256], f8, tag="pwl", name="pwl")
        ones1b = const.tile([1, 128], bf16, tag="ones1b", name="ones1b")

        for t_, d_ in [(wka_t, WKA), (wkc_t, WKC), (wqa_t, WQA), (wqc_t, WQC),
                       (wvb_t, WVB), (bka_t, BKA), (bkc_t, BKC), (bqa_t, BQA),
                       (bqc_t, BQC), (bvr_t, BVR), (rht_t, RHT), (rwt_t, RWT),
                       (pwh_t, PWH), (pwl_t, PWL)]:
            nc.sync.dma_start(t_[:], d_.ap())
        nc.vector.memset(ones1b[:], 1.0)

        KA = big.tile([128, HPG * 8192], f8, tag="ka", name="ka")
        QA = big.tile([128, HPG * 4096], f8, tag="qa", name="qa")
        VN8 = big.tile([128, 32 * 240], f8, tag="vn", name="vn")
        PRJ8 = big.tile([96, 2 * QL], f8, tag="prj", name="prj")

        # one-hot blocks into KA (shared pattern, per-head partition placement)
        for h in range(HPG):
            nc.sync.dma_start(KA[OP[h]:OP[h] + 64, 8192 * h:8192 * h + 8192],
                              IDK.ap())
        vn3 = VN8[:].rearrange("p (kc x) -> p kc x", x=240)
        for h in range(HPG):
            nc.vector.memset(vn3[:, :, 64 + 80 * h], 1.0)

        # ---- Q projection + fp8 hi/lo stores ----
        def emit_qgen(i):
            xt = xp.tile([128, 6 * 512], bf16, tag="xt", name="xq")
            nc.sync.dma_start(
                xt[:].rearrange("p (cb t) -> p cb t", cb=6),
                XTQ.ap().rearrange("p (cb t) -> p cb t", cb=6)[:, :, 512 * i:512 * i + 512])
            psq = p1.tile([128, 512], f32, tag="sc", name="psq")
            for c in range(6):
                nc.tensor.matmul(psq[:], wqa_t[:, 128 * c:128 * c + 128],
                                 xt[:, 512 * c:512 * c + 512],
                                 start=(c == 0), stop=(c == 5))
            psq2 = p1.tile([64, 512], f32, tag="sc", name="psq2")
            for c in range(6):
                nc.tensor.matmul(psq2[:], wqc_t[:, 64 * c:64 * c + 64],
                                 xt[:, 512 * c:512 * c + 512],
                                 start=(c == 0), stop=(c == 5))
            qc, qo = i // 2, 512 * (i % 2)
            for h in range(HPG):
                if h == 0:
                    src, bias = psq[0:64, :], bqa_t[0:64, :]
                elif h == 1:
                    src, bias = psq[64:128, :], bqa_t[64:128, :]
                else:
                    src, bias = psq2[:], bqc_t[:]
                b = KP[h]
                col = 4096 * h + 2048 * qc + qo
                hi = QA[b:b + 64, col:col + 512]
                nc.vector.tensor_scalar(hi, src, bias, None, ADD)
                lo = QA[b:b + 64, col + 1024:col + 1024 + 512]
                nc.vector.scalar_tensor_tensor(lo, src, bias, hi, ADD, SUB)

        # ---- K + V generation for token chunk t (global) ----
        def emit_kv(t):
            xt = xp.tile([128, 6 * 512], bf16, tag="xt", name="xk")
            nc.sync.dma_start(
                xt[:].rearrange("p (cb t) -> p cb t", cb=6),
                XTB.ap().rearrange("p (cb t) -> p cb t", cb=6)[:, :, 512 * t:512 * t + 512])
            psk = p1.tile([128, 512], f32, tag="sc", name="psk")
            for c in range(6):
                nc.tensor.matmul(psk[:], wka_t[:, 128 * c:128 * c + 128],
                                 xt[:, 512 * c:512 * c + 512],
                                 start=(c == 0), stop=(c == 5))
            psk2 = p1.tile([64, 512], f32, tag="sc", name="psk2")
            for c in range(6):
                nc.tensor.matmul(psk2[:], wkc_t[:, 64 * c:64 * c + 64],
                                 xt[:, 512 * c:512 * c + 512],
                                 start=(c == 0), stop=(c == 5))
            for h in range(HPG):
                if h == 0:
                    src, bias = psk[0:64, :], bka_t[0:64, :]
                elif h == 1:
                    src, bias = psk[64:128, :], bka_t[64:128, :]
                else:
                    src, bias = psk2[:], bkc_t[:]
                b = KP[h]
                srcv = src.rearrange("p (kc m) -> p kc m", m=128)
                base = 8192 * h + 1024 * t
                kslab = KA[b:b + 64, base:base + 1024].rearrange(
                    "p (kc two m) -> p kc two m", two=2, m=128)
                for s in range(2):
                    nc.scalar.activation(kslab[:, :, s, :], srcv, AF.Identity,
                                         bias=bias, scale=1.0)
            for sub in range(4):
                pv = p1.tile([128, 192], f32, tag="sc", name="pv")
                for c in range(6):
                    nc.tensor.matmul(pv[:], xt[:, 512 * c + 128 * sub:512 * c + 128 * sub + 128],
                                     wvb_t[:, 192 * c:192 * c + 192],
                                     start=(c == 0), stop=False)
                nc.tensor.matmul(pv[:], ones1b[:], bvr_t[:], start=False, stop=True)
                kc = 4 * t + sub
                vdst = vn3[:, kc, :].rearrange("p (h x) -> p h x", x=80)[:, :, 0:64]
                nc.scalar.activation(vdst, pv[:].rearrange("p (h x) -> p h x", x=64),
                                     AF.Copy, scale=SV)

        # ---- rel-pos generation for head h ----
        def emit_relgen(h):
            b, ob = KP[h], OP[h]
            qh = QA[b:b + 64, 4096 * h:4096 * h + 4096].rearrange(
                "p (qc s q) -> p qc s q", qc=2, s=2)[:, :, 0, :]   # [64, 2, 1024] Q_hi
            for gi in range(4):
                ps = p1.tile([64, 512], f32, tag="sc", name="prh")
                for ii in range(8):
                    i = 8 * gi + ii     # query-row block (64 queries)
                    qcv = qh[:, i // 16, :].rearrange("p (i q) -> p i q", q=64)[:, i % 16, :]
                    nc.tensor.matmul(ps[:, 64 * ii:64 * ii + 64],
                                     rht_t[b:b + 64, 64 * i:64 * i + 64],
                                     qcv, start=True, stop=True)
                col = 4096 * h + 2048 * (gi // 2) + 512 * (gi % 2)
                nc.scalar.activation(QA[ob:ob + 64, col:col + 512], ps[:],
                                     AF.Copy, scale=1.0)
            qw = qh.rearrange("p qc (i w) -> p qc i w", w=64)   # [64, 2, 16, 64]
            for gi in range(4):
                ps = p1.tile([64, 512], f32, tag="sc", name="prw")
                for wi in range(16):
                    w = 16 * gi + wi
                    nc.tensor.matmul(ps[:, 32 * wi:32 * wi + 32],
                                     rwt_t[b:b + 64, 64 * w:64 * w + 64],
                                     qw[:, :, :, w], start=True, stop=True)
                dst = QA[ob:ob + 64, 4096 * h:4096 * h + 4096].rearrange(
                    "p (qc s q) -> p qc s q", qc=2, s=2)[:, :, 1, :].rearrange(
                    "p qc (i w) -> p qc i w", w=64)[:, :, :, 16 * gi:16 * gi + 16]
                nc.scalar.activation(
                    dst, ps[:].rearrange("p (w qc i) -> p qc i w", w=16, qc=2),
                    AF.Copy, scale=1.0)

        # ---- attention stream ----
        O_PS = {}

        def _smat(h, qc, kc):
            S = scp.tile([128, 1024], f32, tag="sc", name="s")
            nc.tensor.matmul(
                S[:],
                KA[:, 8192 * h + 256 * kc:8192 * h + 256 * kc + 256]
                .rearrange("p (two m) -> p two m", two=2),
                QA[:, 4096 * h + 2048 * qc:4096 * h + 2048 * qc + 2048]
                .rearrange("p (two n) -> p two n", two=2),
                start=True, stop=True, perf_mode=DR)
            return S

        AV_Q = []

        def flush_avs():
            while AV_Q:
                O_ps, h, k0, e8, e16a, e16b, gi = AV_Q.pop(0)
                vdr = vn3[:, k0:k0 + 3:2, 80 * h:80 * h + 65]
                nc.tensor.matmul(O_ps[:], vdr,
                                 e8[:].rearrange("p (two n) -> p two n", two=2),
                                 start=(gi == 0), stop=False, perf_mode=DR)
                nc.tensor.matmul(O_ps[:], vn3[:, k0 + 1, 80 * h:80 * h + 65],
                                 e16a[:].bitcast(bf16), start=False, stop=False)
                nc.tensor.matmul(O_ps[:], vn3[:, k0 + 3, 80 * h:80 * h + 65],
                                 e16b[:].bitcast(bf16), start=False,
                                 stop=(gi == 7))

        def emit_group(h, qc, gi):
            """chunks 4gi..4gi+3: ACT on 4gi & 4gi+2 (fp8 + strided DR-AV pair),
            DVE-schraudolph on 4gi+1 & 4gi+3 (bf16 AV). AVs run one group
            behind so they never block the score matmuls feeding the exps."""
            if gi == 0:
                O_PS[(h, qc)] = ovp.tile([65, 1024], f32, tag="ov", name="ov")
            O_ps = O_PS[(h, qc)]
            k0 = 4 * gi
            S0 = _smat(h, qc, k0)
            S1 = _smat(h, qc, k0 + 1)
            e8 = e8p.tile([128, 2048], f8, tag="e8", name="e8")
            nc.scalar.activation(e8[:, 0:1024], S0[:], AF.Exp, scale=1.0 / SEXP)
            e16a = e16p.tile([128, 1024], i16, tag="e16", name="e16a")
            nc.vector.tensor_scalar(e16a[:], S1[:], A_EXP / SEXP, B_EXP, MUL, ADD)
            if taps and h == 0 and qc == 0 and gi == 0:
                sstage = nrm.tile([128, 1024], f32, tag="bsb", name="sstage")
                nc.vector.tensor_copy(sstage[:], S0[:])
                nc.sync.dma_start(TS0.ap(), sstage[:])
            flush_avs()
            S2 = _smat(h, qc, k0 + 2)
            S3 = _smat(h, qc, k0 + 3)
            nc.scalar.activation(e8[:, 1024:2048], S2[:], AF.Exp, scale=1.0 / SEXP)
            e16b = e16p.tile([128, 1024], i16, tag="e16", name="e16b")
            nc.vector.tensor_scalar(e16b[:], S3[:], A_EXP / SEXP, B_EXP, MUL, ADD)
            if taps and h == 0 and qc == 0 and gi == 0:
                nc.sync.dma_start(TE8.ap(), e8[:])
            AV_Q.append((O_ps, h, k0, e8, e16a, e16b, gi))

        # ---- normalization: PRJ8 = (SP/SV) * O/d, fp8 ----
        RECTS = [  # (head, prj_part0, prj_slot, o_row0, nrows)
            (0, 0, 0, 0, 64),
            (1, 64, 0, 0, 32), (1, 0, 1, 32, 32),
            (2, 32, 1, 0, 64),
        ]

        def emit_norm(h, qc):
            flush_avs()
            O_ps = O_PS.pop((h, qc))
            rec = nrm.tile([1, 1024], f32, tag="rec", name="rec")
            nc.vector.reciprocal_approx_fast(rec[:], O_ps[64:65, :])
            bsb = nrm.tile([128, 1024], f32, tag="bsb", name="bsb")
            nc.gpsimd.partition_broadcast(bsb[:], rec[:])
            prjv = PRJ8[:].rearrange("p (s q) -> p s q", s=2)
            for (hh, p0, sl, o0, nr) in RECTS:
                if hh != h:
                    continue
                dst = prjv[p0:p0 + nr, sl, 1024 * qc:1024 * qc + 1024]
                nc.vector.scalar_tensor_tensor(
                    dst, O_ps[o0:o0 + nr, :], SP / SV, bsb[o0:o0 + nr, :],
                    MUL, MUL)

        # ---- output projection (fp8-DR hi/lo), DMA from PSUM ----
        def emit_proj(qc, m, wh):
            pp = p1.tile([128, 512], f32, tag="sc", name="pp")
            mv = PRJ8[:].rearrange("p (s q) -> p s q", s=2)[
                :, :, 1024 * qc + 512 * wh:1024 * qc + 512 * wh + 512]
            nc.tensor.matmul(pp[:], pwh_t[:, 256 * m:256 * m + 256]
                             .rearrange("p (two c) -> p two c", two=2),
                             mv, start=True, stop=False, perf_mode=DR)
            nc.tensor.matmul(pp[:], pwl_t[:, 256 * m:256 * m + 256]
                             .rearrange("p (two c) -> p two c", two=2),
                             mv, start=False, stop=True, perf_mode=DR)
            ost = stg.tile([128, 512], bf16, tag="ost", name="ost")
            if (2 * m + wh) % 2 == 0:
                nc.vector.tensor_copy(ost[:], pp[:])
            else:
                nc.scalar.activation(ost[:], pp[:], AF.Copy)
            nc.sync.dma_start(
                OUT.ap()[128 * m:128 * m + 128,
                         1024 * qc + 512 * wh:1024 * qc + 512 * wh + 512], ost[:])

        # ================= schedule =================
        for i in range(4):
            emit_qgen(i)
        emit_relgen(0)
        # (0,0) stream chases K/V generation (group gi consumes t-chunk gi)
        emit_kv(0)
        emit_kv(1)
        for gi in range(8):
            emit_group(0, 0, gi)
            if gi < 6:
                emit_kv(gi + 2)
        emit_norm(0, 0)
        emit_relgen(1)
        for gi in range(8):
            emit_group(0, 1, gi)
        emit_norm(0, 1)
        for gi in range(8):
            emit_group(1, 0, gi)
        emit_norm(1, 0)
        emit_relgen(2)
        for gi in range(8):
            emit_group(1, 1, gi)
        emit_norm(1, 1)
        for gi in range(8):
            emit_group(2, 0, gi)
        emit_norm(2, 0)
        for gi in range(8):
            emit_group(2, 1, gi)
            # hide qc0 output projection under the last attention segment
            if 1 <= gi <= 6:
                m = gi - 1
                emit_proj(0, m, 0)
                emit_proj(0, m, 1)
        emit_norm(2, 1)

        if taps:
            nc.sync.dma_start(TKA.ap(), KA[:])
            nc.sync.dma_start(TQA.ap(), QA[:])
            nc.sync.dma_start(TVN.ap(), VN8[:])
            nc.sync.dma_start(TPRJ.ap(), PRJ8[:])

        for m in range(6):
            emit_proj(1, m, 0)
            emit_proj(1, m, 1)

    nc.compile()
    return nc


def _host_inputs(x, qkv_w, qkv_b, proj_w, rel_pos_h, rel_pos_w):
    xm = np.ascontiguousarray(x.reshape(N, C)).astype(np.float32)
    xT = xm.T  # (C, N)
    xtb = np.ascontiguousarray(
        xT.reshape(6, 128, N).transpose(1, 0, 2).reshape(128, 6 * N)).astype(BF)

    idx = np.arange(64)[:, None] - np.arange(64)[None, :] + 63
    rh_g = rel_pos_h[idx] * (SREL / SQ)   # (hrow, kh, c)
    rw_g = rel_pos_w[idx] * (SREL / SQ)   # (w, kw, c)
    rwt1 = np.ascontiguousarray(
        rw_g.transpose(2, 0, 1).reshape(64, 64 * 64)).astype(BF)
    rwt = np.concatenate([rwt1, rwt1], axis=0)  # dup rows -> [128, 4096]

    # one-hot block [64, 32*2*128]
    kcs = np.arange(32)
    m = np.arange(128)
    jj = np.arange(64)
    idkh = (jj[:, None, None] == (2 * kcs[None, :, None] + m[None, None, :] // 64))
    idkw = (jj[:, None, None] == (m[None, None, :] % 64))[:, [0] * 32, :] \
        if False else np.broadcast_to(
            (jj[:, None] == (m[None, :] % 64))[:, None, :], (64, 32, 128))
    idk = np.zeros((64, 32, 2, 128), dtype=np.float32)
    idk[:, :, 0, :] = idkh
    idk[:, :, 1, :] = idkw
    idk = np.ascontiguousarray(idk.reshape(64, 32 * 256)).astype(F8)

    in_maps = []
    for core in range(8):
        g, j = core // QB, core % QB
        cs = slice(192 * g, 192 * g + 192)
        wq = qkv_w[:, 0 * C:1 * C][:, cs] * SQ
        wk = qkv_w[:, 1 * C:2 * C][:, cs] * (SCALE * SK)
        wv = qkv_w[:, 2 * C:3 * C][:, cs]
        bq = qkv_b[0 * C:1 * C][cs] * SQ
        bk = qkv_b[1 * C:2 * C][cs] * (SCALE * SK)
        bv = qkv_b[2 * C:3 * C][cs]

        h0 = 32 * j
        rht1 = np.ascontiguousarray(
            rh_g[h0:h0 + 32].transpose(2, 0, 1).reshape(64, 32 * 64)).astype(BF)
        rht = np.concatenate([rht1, rht1], axis=0)

        pw = proj_w[cs] * (SW / SP)        # (192, 768)
        pw96 = pw.reshape(2, 96, 768).transpose(1, 0, 2)   # [96, s, 768]
        pwp = np.ascontiguousarray(
            pw96.reshape(96, 2, 6, 128).transpose(0, 2, 1, 3).reshape(96, 6 * 256))
        pwh = pwp.astype(F8)
        pwl = (pwp - pwh.astype(np.float32)).astype(F8)

        xtq = np.ascontiguousarray(
            xT[:, QL * j:QL * j + QL].reshape(6, 128, QL)
            .transpose(1, 0, 2).reshape(128, 6 * QL)).astype(BF)

        in_maps.append({
            "xtb": xtb,
            "xtq": xtq,
            "wka": _pack6(wk[:, 0:128]).astype(BF),
            "wkc": _pack6(wk[:, 128:192]).astype(BF),
            "wqa": _pack6(wq[:, 0:128]).astype(BF),
            "wqc": _pack6(wq[:, 128:192]).astype(BF),
            "wvb": _pack6(wv).astype(BF),
            "bka": np.ascontiguousarray(bk[0:128, None].astype(np.float32)),
            "bkc": np.ascontiguousarray(bk[128:192, None].astype(np.float32)),
            "bqa": np.ascontiguousarray(bq[0:128, None].astype(np.float32)),
            "bqc": np.ascontiguousarray(bq[128:192, None].astype(np.float32)),
            "bvr": np.ascontiguousarray(bv[None, :]).astype(BF),
            "rht": rht,
            "rwt": rwt,
            "idk": idk,
            "pwh": pwh,
            "pwl": pwl,
        })
    return in_maps


def kernel(x, qkv_w, qkv_b, proj_w, proj_b, rel_pos_h, rel_pos_w):
    from concourse.bass_utils import run_bass_kernel_spmd

    x = np.asarray(x, dtype=np.float32)
    qkv_w = np.asarray(qkv_w, dtype=np.float32)
    qkv_b = np.asarray(qkv_b, dtype=np.float32)
    proj_w = np.asarray(proj_w, dtype=np.float32)
    proj_b = np.asarray(proj_b, dtype=np.float32)
    rel_pos_h = np.asarray(rel_pos_h, dtype=np.float32)
    rel_pos_w = np.asarray(rel_pos_w, dtype=np.float32)

    if "nc" not in _prog_cache:
        _prog_cache["nc"] = _build_program()
    nc = _prog_cache["nc"]

    in_maps = _host_inputs(x, qkv_w, qkv_b, proj_w, rel_pos_h, rel_pos_w)
    res = run_bass_kernel_spmd(nc, in_maps, core_ids=list(range(8)))

    out = np.zeros((N, C), dtype=np.float32)
    for core in range(8):
        g, j = core // QB, core % QB
        out[QL * j:QL * j + QL, :] += res.results[core]["out"].astype(np.float32).T / SW
    out += proj_b[None, :]
    return out.reshape(1, H, W, C).astype(np.float32)


# revision 22
# speedup vs baseline: 1.4061x; 1.0393x over previous
"""Trainium2 Bass kernel for ViT-style attention with decomposed relative
position bias (fp8 DoubleRow edition).

Problem: x(1,64,64,768) -> qkv proj -> 12-head attention with rel_pos_h/w
decomposed bias -> softmax -> out proj.  N=4096 tokens, hd=64.

Sharding: 8 cores = 4 head-groups (3 heads) x 2 query-blocks (2048 queries).

Per-core design (all matmul-heavy paths in fp8 DoubleRow at 0.5 cyc/row):
- Scores: ONE fp8-DR matmul per 128-key chunk folds everything:
    stationary slots  p0-63:(K,K)dup   p64-127:(IDKH,IDKW) one-hots
    moving   slots  p0-63:(Qhi,Qlo)  p64-127:(RH,RW)
  => S = K*(Qhi+Qlo) + rel_h + rel_w, K 8-bit/Q 12-bit, pre-scaled by
  SK*SQ=64 (fp8 range), un-scaled inside exp.
- exp split across engines: even kc-pairs on ACT (true exp -> fp8 E, AV is a
  fp8-DR pair matmul), odd pairs on DVE via Schraudolph bit-trick exp
  (int16 = S*a+b, bitcast bf16; AV is bf16 matmul on fp8 V stationary).
- Softmax denominators from a ones-column in the V stationary; 1/d via
  reciprocal_approx_fast (DVE), broadcast on GPSIMD, fused normalize-mul
  into fp8 PRJ tile (DVE).
- K/Q/V generation in bf16 (exact-ish); K/V/rel converts on ACT
  (Identity/Copy activations share the exp table set -> no table reloads);
  output projection as fp8-DR (hi/lo weights), DMA'd straight from PSUM.
- Head-alternating partition layout (h0:p0-63, h1:p64-127, h2:p0-63 for
  K/Q; one-hots/rel on the complement) so gen-PSUM rows map 1:1.
"""

import numpy as np
import ml_dtypes

NH, HD, C, H, W = 12, 64, 768, 64, 64
N = H * W            # 4096
G, QB = 4, 2         # head groups x query blocks = 8 cores
HPG = NH // G        # 3 heads per group
QL = N // QB         # 2048 queries per block
SCALE = HD ** -0.5

SK, SQ, SREL, SV = 16.0, 4.0, 64.0, 16.0
SEXP = SK * SQ       # scores arrive in PSUM scaled by 64
SP = 256.0           # PRJ tile scale (PRJ = SP * O/d)
SW = 8192.0          # PSUM proj-out scale (host divides)
A_EXP = 184.6650390625
B_EXP = 16250.35

F8 = ml_dtypes.float8_e4m3
BF = ml_dtypes.bfloat16

KP = (0, 64, 0)      # K/Q base partition per local head
OP = (64, 0, 64)     # one-hot / rel base partition

_prog_cache = {}


def _pack6(w):
    # (768, M) -> [128, 6*M]: contraction chunk c of 128 at cols [c*M:(c+1)*M]
    m = w.shape[1]
    return np.ascontiguousarray(
        w.reshape(6, 128, m).transpose(1, 0, 2).reshape(128, 6 * m))


def _build_program(taps=False):
    import concourse.bacc as bacc
    import concourse.mybir as mybir
    import concourse.tile as tile
    from contextlib import ExitStack

    f32 = mybir.dt.float32
    bf16 = mybir.dt.bfloat16
    i16 = mybir.dt.int16
    f8 = mybir.dt.float8e4
    AF = mybir.ActivationFunctionType
    ADD = mybir.AluOpType.add
    SUB = mybir.AluOpType.subtract
    MUL = mybir.AluOpType.mult
    DR = mybir.MatmulPerfMode.DoubleRow

    nc = bacc.Bacc("TRN2", target_bir_lowering=False, debug=False)

    XTB = nc.dram_tensor("xtb", [128, 6 * N], bf16, kind="ExternalInput")
    XTQ = nc.dram_tensor("xtq", [128, 6 * QL], bf16, kind="ExternalInput")
    WKA = nc.dram_tensor("wka", [128, 6 * 128], bf16, kind="ExternalInput")
    WKC = nc.dram_tensor("wkc", [128, 6 * 64], bf16, kind="ExternalInput")
    WQA = nc.dram_tensor("wqa", [128, 6 * 128], bf16, kind="ExternalInput")
    WQC = nc.dram_tensor("wqc", [128, 6 * 64], bf16, kind="ExternalInput")
    WVB = nc.dram_tensor("wvb", [128, 6 * 192], bf16, kind="ExternalInput")
    BKA = nc.dram_tensor("bka", [128, 1], f32, kind="ExternalInput")
    BKC = nc.dram_tensor("bkc", [64, 1], f32, kind="ExternalInput")
    BQA = nc.dram_tensor("bqa", [128, 1], f32, kind="ExternalInput")
    BQC = nc.dram_tensor("bqc", [64, 1], f32, kind="ExternalInput")
    BVR = nc.dram_tensor("bvr", [1, 192], bf16, kind="ExternalInput")
    RHT = nc.dram_tensor("rht", [128, 32 * 64], bf16, kind="ExternalInput")
    RWT = nc.dram_tensor("rwt", [128, 64 * 64], bf16, kind="ExternalInput")
    IDK = nc.dram_tensor("idk", [64, 32 * 256], f8, kind="ExternalInput")
    ZR = nc.dram_tensor("zr", [64, 4096], f8, kind="ExternalInput")
    PWH = nc.dram_tensor("pwh", [96, 6 * 256], f8, kind="ExternalInput")
    PWL = nc.dram_tensor("pwl", [96, 6 * 256], f8, kind="ExternalInput")
    OUT = nc.dram_tensor("out", [C, QL], bf16, kind="ExternalOutput")

    if taps:
        TKA = nc.dram_tensor("t_ka", [128, HPG * 8192], f8, kind="ExternalOutput")
        TQA = nc.dram_tensor("t_qa", [128, HPG * 4096], f8, kind="ExternalOutput")
        TVN = nc.dram_tensor("t_vn", [128, 32 * 240], f8, kind="ExternalOutput")
        TS0 = nc.dram_tensor("t_s0", [128, 1024], f32, kind="ExternalOutput")
        TE8 = nc.dram_tensor("t_e8", [128, 2048], f8, kind="ExternalOutput")
        TPRJ = nc.dram_tensor("t_prj", [96, 4096], f8, kind="ExternalOutput")

    with tile.TileContext(nc) as tc, ExitStack() as es:
        const = es.enter_context(tc.tile_pool(name="const", bufs=1))
        big = es.enter_context(tc.tile_pool(name="big", bufs=1))
        xp = es.enter_context(tc.tile_pool(name="xp", bufs=2))
        scp = es.enter_context(tc.tile_pool(name="sc", bufs=3, space="PSUM"))
        p1 = scp
        ovp = es.enter_context(tc.tile_pool(name="ov", bufs=1, space="PSUM"))
        e8p = es.enter_context(tc.tile_pool(name="e8p", bufs=2))
        e16p = es.enter_context(tc.tile_pool(name="e16p", bufs=4))
        nrm = es.enter_context(tc.tile_pool(name="nrm", bufs=2))
        stg = es.enter_context(tc.tile_pool(name="stg", bufs=3))

        # ---- persistent tiles ----
        wka_t = const.tile([128, 6 * 128], bf16, tag="wka", name="wka")
        wkc_t = const.tile([128, 6 * 64], bf16, tag="wkc", name="wkc")
        wqa_t = const.tile([128, 6 * 128], bf16, tag="wqa", name="wqa")
        wqc_t = const.tile([128, 6 * 64], bf16, tag="wqc", name="wqc")
        wvb_t = const.tile([128, 6 * 192], bf16, tag="wvb", name="wvb")
        bka_t = const.tile([128, 1], f32, tag="bka", name="bka")
        bkc_t = const.tile([64, 1], f32, tag="bkc", name="bkc")
        bqa_t = const.tile([128, 1], f32, tag="bqa", name="bqa")
        bqc_t = const.tile([64, 1], f32, tag="bqc", name="bqc")
        bvr_t = const.tile([1, 192], bf16, tag="bvr", name="bvr")
        rht_t = const.tile([128, 32 * 64], bf16, tag="rht", name="rht")
        rwt_t = const.tile([128, 64 * 64], bf16, tag="rwt", name="rwt")
        pwh_t = const.tile([96, 6 * 256], f8, tag="pwh", name="pwh")
        pwl_t = const.tile([96, 6 * 256], f8, tag="pwl", name="pwl")
        ones1b = const.tile([1, 128], bf16, tag="ones1b", name="ones1b")

        for t_, d_ in [(wqa_t, WQA), (wqc_t, WQC), (bqa_t, BQA), (bqc_t, BQC),
                       (wka_t, WKA), (wkc_t, WKC), (bka_t, BKA), (bkc_t, BKC),
                       (wvb_t, WVB), (bvr_t, BVR)]:
            nc.sync.dma_start(t_[:], d_.ap())
        nc.vector.memset(ones1b[:], 1.0)

        KA = big.tile([128, HPG * 8192], f8, tag="ka", name="ka")
        QA = big.tile([128, HPG * 4096], f8, tag="qa", name="qa")
        VN8 = big.tile([128, 32 * 240], f8, tag="vn", name="vn")
        PRJ8 = big.tile([96, 2 * QL], f8, tag="prj", name="prj")

        def emit_late_consts():
            # one-hot blocks into KA (shared pattern, per-head placement);
            # K slot1 and Q slot1 zeroed (single-fp8 K and Q)
            for t_, d_ in [(rht_t, RHT), (rwt_t, RWT)]:
                nc.sync.dma_start(t_[:], d_.ap())
            for h in range(HPG):
                nc.sync.dma_start(KA[OP[h]:OP[h] + 64, 8192 * h:8192 * h + 8192],
                                  IDK.ap())
                nc.sync.dma_start(
                    KA[KP[h]:KP[h] + 64, 8192 * h:8192 * h + 8192].rearrange(
                        "p (kc two m) -> p kc two m", two=2, m=128)[:, :, 1, :],
                    ZR.ap().rearrange("p (kc m) -> p kc m", m=128))
                nc.sync.dma_start(
                    QA[KP[h]:KP[h] + 64, 4096 * h:4096 * h + 4096].rearrange(
                        "p (qc s q) -> p qc s q", qc=2, s=2)[:, :, 1, :],
                    ZR.ap()[:, 0:2048].rearrange("p (qc q) -> p qc q", qc=2))
            for t_, d_ in [(pwh_t, PWH), (pwl_t, PWL)]:
                nc.sync.dma_start(t_[:], d_.ap())

        vn3 = VN8[:].rearrange("p (kc x) -> p kc x", x=240)
        for h in range(HPG):
            nc.vector.memset(vn3[:, :, 64 + 80 * h], 1.0)

        # ---- Q projection + fp8 hi/lo stores ----
        def emit_qgen(i):
            xt = xp.tile([128, 6 * 512], bf16, tag="xt", name="xq")
            nc.sync.dma_start(
                xt[:].rearrange("p (cb t) -> p cb t", cb=6),
                XTQ.ap().rearrange("p (cb t) -> p cb t", cb=6)[:, :, 512 * i:512 * i + 512])
            psq = p1.tile([128, 512], f32, tag="sc", name="psq")
            for c in range(6):
                nc.tensor.matmul(psq[:], wqa_t[:, 128 * c:128 * c + 128],
                                 xt[:, 512 * c:512 * c + 512],
                                 start=(c == 0), stop=(c == 5))
            psq2 = p1.tile([64, 512], f32, tag="sc", name="psq2")
            for c in range(6):
                nc.tensor.matmul(psq2[:], wqc_t[:, 64 * c:64 * c + 64],
                                 xt[:, 512 * c:512 * c + 512],
                                 start=(c == 0), stop=(c == 5))
            qc, qo = i // 2, 512 * (i % 2)
            for h in range(HPG):
                if h == 0:
                    src, bias = psq[0:64, :], bqa_t[0:64, :]
                elif h == 1:
                    src, bias = psq[64:128, :], bqa_t[64:128, :]
                else:
                    src, bias = psq2[:], bqc_t[:]
                b = KP[h]
                col = 4096 * h + 2048 * qc + qo
                hi = QA[b:b + 64, col:col + 512]
                nc.vector.tensor_scalar(hi, src, bias, None, ADD)

        # ---- K + V generation for token chunk t (global) ----
        def emit_kv(t):
            xt = xp.tile([128, 6 * 512], bf16, tag="xt", name="xk")
            nc.sync.dma_start(
                xt[:].rearrange("p (cb t) -> p cb t", cb=6),
                XTB.ap().rearrange("p (cb t) -> p cb t", cb=6)[:, :, 512 * t:512 * t + 512])
            psk = p1.tile([128, 512], f32, tag="sc", name="psk")
            for c in range(6):
                nc.tensor.matmul(psk[:], wka_t[:, 128 * c:128 * c + 128],
                                 xt[:, 512 * c:512 * c + 512],
                                 start=(c == 0), stop=(c == 5))
            psk2 = p1.tile([64, 512], f32, tag="sc", name="psk2")
            for c in range(6):
                nc.tensor.matmul(psk2[:], wkc_t[:, 64 * c:64 * c + 64],
                                 xt[:, 512 * c:512 * c + 512],
                                 start=(c == 0), stop=(c == 5))
            for h in range(HPG):
                if h == 0:
                    src, bias = psk[0:64, :], bka_t[0:64, :]
                elif h == 1:
                    src, bias = psk[64:128, :], bka_t[64:128, :]
                else:
                    src, bias = psk2[:], bkc_t[:]
                b = KP[h]
                srcv = src.rearrange("p (kc m) -> p kc m", m=128)
                base = 8192 * h + 1024 * t
                kslab = KA[b:b + 64, base:base + 1024].rearrange(
                    "p (kc two m) -> p kc two m", two=2, m=128)
                nc.scalar.activation(kslab[:, :, 0, :], srcv, AF.Identity,
                                     bias=bias, scale=1.0)
            for sub in range(4):
                pv = p1.tile([128, 192], f32, tag="sc", name="pv")
                for c in range(6):
                    nc.tensor.matmul(pv[:], xt[:, 512 * c + 128 * sub:512 * c + 128 * sub + 128],
                                     wvb_t[:, 192 * c:192 * c + 192],
                                     start=(c == 0), stop=False)
                nc.tensor.matmul(pv[:], ones1b[:], bvr_t[:], start=False, stop=True)
                kc = 4 * t + sub
                vdst = vn3[:, kc, :].rearrange("p (h x) -> p h x", x=80)[:, :, 0:64]
                nc.scalar.activation(vdst, pv[:].rearrange("p (h x) -> p h x", x=64),
                                     AF.Copy, scale=SV)

        # ---- rel-pos generation for head h ----
        def emit_relgen(h):
            b, ob = KP[h], OP[h]
            qh = QA[b:b + 64, 4096 * h:4096 * h + 4096].rearrange(
                "p (qc s q) -> p qc s q", qc=2, s=2)[:, :, 0, :]   # [64, 2, 1024] Q_hi
            for gi in range(4):
                ps = p1.tile([64, 512], f32, tag="sc", name="prh")
                for ii in range(8):
                    i = 8 * gi + ii     # query-row block (64 queries)
                    qcv = qh[:, i // 16, :].rearrange("p (i q) -> p i q", q=64)[:, i % 16, :]
                    nc.tensor.matmul(ps[:, 64 * ii:64 * ii + 64],
                                     rht_t[b:b + 64, 64 * i:64 * i + 64],
                                     qcv, start=True, stop=True)
                col = 4096 * h + 2048 * (gi // 2) + 512 * (gi % 2)
                nc.scalar.activation(QA[ob:ob + 64, col:col + 512], ps[:],
                                     AF.Copy, scale=1.0)
            qw = qh.rearrange("p qc (i w) -> p qc i w", w=64)   # [64, 2, 16, 64]
            for gi in range(4):
                ps = p1.tile([64, 512], f32, tag="sc", name="prw")
                for wi in range(16):
                    w = 16 * gi + wi
                    nc.tensor.matmul(ps[:, 32 * wi:32 * wi + 32],
                                     rwt_t[b:b + 64, 64 * w:64 * w + 64],
                                     qw[:, :, :, w], start=True, stop=True)
                dst = QA[ob:ob + 64, 4096 * h:4096 * h + 4096].rearrange(
                    "p (qc s q) -> p qc s q", qc=2, s=2)[:, :, 1, :].rearrange(
                    "p qc (i w) -> p qc i w", w=64)[:, :, :, 16 * gi:16 * gi + 16]
                nc.scalar.activation(
                    dst, ps[:].rearrange("p (w qc i) -> p qc i w", w=16, qc=2),
                    AF.Copy, scale=1.0)

        # ---- attention stream ----
        O_PS = {}

        def _smat(h, qc, kc):
            S = scp.tile([128, 1024], f32, tag="sc", name="s")
            nc.tensor.matmul(
                S[:],
                KA[:, 8192 * h + 256 * kc:8192 * h + 256 * kc + 256]
                .rearrange("p (two m) -> p two m", two=2),
                QA[:, 4096 * h + 2048 * qc:4096 * h + 2048 * qc + 2048]
                .rearrange("p (two n) -> p two n", two=2),
                start=True, stop=True, perf_mode=DR)
            return S

        AV_Q = []

        def flush_avs():
            while AV_Q:
                O_ps, h, k0, e8, e16a, e16b, gi = AV_Q.pop(0)
                vdr = vn3[:, k0:k0 + 3:2, 80 * h:80 * h + 65]
                nc.tensor.matmul(O_ps[:], vdr,
                                 e8[:].rearrange("p (two n) -> p two n", two=2),
                                 start=(gi == 0), stop=False, perf_mode=DR)
                nc.tensor.matmul(O_ps[:], vn3[:, k0 + 1, 80 * h:80 * h + 65],
                                 e16a[:].bitcast(bf16), start=False, stop=False)
                nc.tensor.matmul(O_ps[:], vn3[:, k0 + 3, 80 * h:80 * h + 65],
                                 e16b[:].bitcast(bf16), start=False,
                                 stop=(gi == 7))

        def emit_group(h, qc, gi):
            """chunks 4gi..4gi+3: ACT on 4gi & 4gi+2 (fp8 + strided DR-AV pair),
            DVE-schraudolph on 4gi+1 & 4gi+3 (bf16 AV). AVs run one group
            behind so they never block the score matmuls feeding the exps."""
            if gi == 0:
                O_PS[(h, qc)] = ovp.tile([65, 1024], f32, tag="ov", name="ov")
            O_ps = O_PS[(h, qc)]
            k0 = 4 * gi
            S0 = _smat(h, qc, k0)
            S1 = _smat(h, qc, k0 + 1)
            e8 = e8p.tile([128, 2048], f8, tag="e8", name="e8")
            nc.scalar.activation(e8[:, 0:1024], S0[:], AF.Exp, scale=1.0 / SEXP)
            e16a = e16p.tile([128, 1024], i16, tag="e16", name="e16a")
            nc.vector.tensor_scalar(e16a[:], S1[:], A_EXP / SEXP, B_EXP, MUL, ADD)
            if taps and h == 0 and qc == 0 and gi == 0:
                sstage = nrm.tile([128, 1024], f32, tag="bsb", name="sstage")
                nc.vector.tensor_copy(sstage[:], S0[:])
                nc.sync.dma_start(TS0.ap(), sstage[:])
            flush_avs()
            S2 = _smat(h, qc, k0 + 2)
            S3 = _smat(h, qc, k0 + 3)
            nc.scalar.activation(e8[:, 1024:2048], S2[:], AF.Exp, scale=1.0 / SEXP)
            e16b = e16p.tile([128, 1024], i16, tag="e16", name="e16b")
            nc.vector.tensor_scalar(e16b[:], S3[:], A_EXP / SEXP, B_EXP, MUL, ADD)
            if taps and h == 0 and qc == 0 and gi == 0:
                nc.sync.dma_start(TE8.ap(), e8[:])
            AV_Q.append((O_ps, h, k0, e8, e16a, e16b, gi))

        # ---- normalization: PRJ8 = (SP/SV) * O/d, fp8 ----
        RECTS = [  # (head, prj_part0, prj_slot, o_row0, nrows)
            (0, 0, 0, 0, 64),
            (1, 64, 0, 0, 32), (1, 0, 1, 32, 32),
            (2, 32, 1, 0, 32), (2, 64, 1, 32, 32),
        ]

        def emit_norm(h, qc):
            flush_avs()
            O_ps = O_PS.pop((h, qc))
            rec = nrm.tile([1, 1024], f32, tag="rec", name="rec")
            nc.vector.reciprocal_approx_fast(rec[:], O_ps[64:65, :])
            bsb = nrm.tile([128, 1024], f32, tag="bsb", name="bsb")
            nc.gpsimd.partition_broadcast(bsb[:], rec[:])
            prjv = PRJ8[:].rearrange("p (s q) -> p s q", s=2)
            for (hh, p0, sl, o0, nr) in RECTS:
                if hh != h:
                    continue
                dst = prjv[p0:p0 + nr, sl, 1024 * qc:1024 * qc + 1024]
                nc.vector.scalar_tensor_tensor(
                    dst, O_ps[o0:o0 + nr, :], SP / SV, bsb[o0:o0 + nr, :],
                    MUL, MUL)

        # ---- output projection (fp8-DR hi/lo), DMA from PSUM ----
        def emit_proj(qc, m, wh):
            pp = p1.tile([128, 512], f32, tag="sc", name="pp")
            mv = PRJ8[:].rearrange("p (s q) -> p s q", s=2)[
                :, :, 1024 * qc + 512 * wh:1024 * qc + 512 * wh + 512]
            nc.tensor.matmul(pp[:], pwh_t[:, 256 * m:256 * m + 256]
                             .rearrange("p (two c) -> p two c", two=2),
                             mv, start=True, stop=False, perf_mode=DR)
            nc.tensor.matmul(pp[:], pwl_t[:, 256 * m:256 * m + 256]
                             .rearrange("p (two c) -> p two c", two=2),
                             mv, start=False, stop=True, perf_mode=DR)
            ost = stg.tile([128, 512], bf16, tag="ost", name="ost")
            if (2 * m + wh) % 2 == 0:
                nc.vector.tensor_copy(ost[:], pp[:])
            else:
                nc.scalar.activation(ost[:], pp[:], AF.Copy)
            nc.sync.dma_start(
                OUT.ap()[128 * m:128 * m + 128,
                         1024 * qc + 512 * wh:1024 * qc + 512 * wh + 512], ost[:])

        # ================= schedule =================
        for i in range(4):
            emit_qgen(i)
        emit_late_consts()
        emit_relgen(0)
        # (0,0) stream chases K/V generation (group gi consumes t-chunk gi)
        emit_kv(0)
        emit_kv(1)
        for gi in range(8):
            emit_group(0, 0, gi)
            if gi < 6:
                emit_kv(gi + 2)
        emit_norm(0, 0)
        emit_relgen(1)
        for gi in range(8):
            emit_group(0, 1, gi)
        emit_norm(0, 1)
        for gi in range(8):
            emit_group(1, 0, gi)
        emit_norm(1, 0)
        emit_relgen(2)
        for gi in range(8):
            emit_group(1, 1, gi)
        emit_norm(1, 1)
        for gi in range(8):
            emit_group(2, 0, gi)
        emit_norm(2, 0)
        for gi in range(8):
            emit_group(2, 1, gi)
            # hide qc0 output projection under the last attention segment
            if 1 <= gi <= 6:
                m = gi - 1
                emit_proj(0, m, 0)
                emit_proj(0, m, 1)
        emit_norm(2, 1)

        if taps:
            nc.sync.dma_start(TKA.ap(), KA[:])
            nc.sync.dma_start(TQA.ap(), QA[:])
            nc.sync.dma_start(TVN.ap(), VN8[:])
            nc.sync.dma_start(TPRJ.ap(), PRJ8[:])

        for m in range(6):
            emit_proj(1, m, 0)
            emit_proj(1, m, 1)

    nc.compile()
    return nc


def _host_inputs(x, qkv_w, qkv_b, proj_w, rel_pos_h, rel_pos_w):
    xm = np.ascontiguousarray(x.reshape(N, C)).astype(np.float32)
    xT = xm.T  # (C, N)
    xtb = np.ascontiguousarray(
        xT.reshape(6, 128, N).transpose(1, 0, 2).reshape(128, 6 * N)).astype(BF)

    idx = np.arange(64)[:, None] - np.arange(64)[None, :] + 63
    rh_g = rel_pos_h[idx] * (SREL / SQ)   # (hrow, kh, c)
    rw_g = rel_pos_w[idx] * (SREL / SQ)   # (w, kw, c)
    rwt1 = np.ascontiguousarray(
        rw_g.transpose(2, 0, 1).reshape(64, 64 * 64)).astype(BF)
    rwt = np.concatenate([rwt1, rwt1], axis=0)  # dup rows -> [128, 4096]

    # one-hot block [64, 32*2*128]
    kcs = np.arange(32)
    m = np.arange(128)
    jj = np.arange(64)
    idkh = (jj[:, None, None] == (2 * kcs[None, :, None] + m[None, None, :] // 64))
    idkw = (jj[:, None, None] == (m[None, None, :] % 64))[:, [0] * 32, :] \
        if False else np.broadcast_to(
            (jj[:, None] == (m[None, :] % 64))[:, None, :], (64, 32, 128))
    idk = np.zeros((64, 32, 2, 128), dtype=np.float32)
    idk[:, :, 0, :] = idkh
    idk[:, :, 1, :] = idkw
    idk = np.ascontiguousarray(idk.reshape(64, 32 * 256)).astype(F8)

    in_maps = []
    for core in range(8):
        g, j = core // QB, core % QB
        cs = slice(192 * g, 192 * g + 192)
        wq = qkv_w[:, 0 * C:1 * C][:, cs] * SQ
        wk = qkv_w[:, 1 * C:2 * C][:, cs] * (SCALE * SK)
        wv = qkv_w[:, 2 * C:3 * C][:, cs]
        bq = qkv_b[0 * C:1 * C][cs] * SQ
        bk = qkv_b[1 * C:2 * C][cs] * (SCALE * SK)
        bv = qkv_b[2 * C:3 * C][cs]

        h0 = 32 * j
        rht1 = np.ascontiguousarray(
            rh_g[h0:h0 + 32].transpose(2, 0, 1).reshape(64, 32 * 64)).astype(BF)
        rht = np.concatenate([rht1, rht1], axis=0)

        pw = proj_w[cs] * (SW / SP)        # (192, 768)
        pw96 = pw.reshape(2, 96, 768).transpose(1, 0, 2)   # [96, s, 768]
        pwp = np.ascontiguousarray(
            pw96.reshape(96, 2, 6, 128).transpose(0, 2, 1, 3).reshape(96, 6 * 256))
        pwh = pwp.astype(F8)
        pwl = (pwp - pwh.astype(np.float32)).astype(F8)

        xtq = np.ascontiguousarray(
            xT[:, QL * j:QL * j + QL].reshape(6, 128, QL)
            .transpose(1, 0, 2).reshape(128, 6 * QL)).astype(BF)

        in_maps.append({
            "xtb": xtb,
            "xtq": xtq,
            "wka": _pack6(wk[:, 0:128]).astype(BF),
            "wkc": _pack6(wk[:, 128:192]).astype(BF),
            "wqa": _pack6(wq[:, 0:128]).astype(BF),
            "wqc": _pack6(wq[:, 128:192]).astype(BF),
            "wvb": _pack6(wv).astype(BF),
            "bka": np.ascontiguousarray(bk[0:128, None].astype(np.float32)),
            "bkc": np.ascontiguousarray(bk[128:192, None].astype(np.float32)),
            "bqa": np.ascontiguousarray(bq[0:128, None].astype(np.float32)),
            "bqc": np.ascontiguousarray(bq[128:192, None].astype(np.float32)),
            "bvr": np.ascontiguousarray(bv[None, :]).astype(BF),
            "rht": rht,
            "rwt": rwt,
            "idk": idk,
            "zr": np.zeros((64, 4096), dtype=F8),
            "pwh": pwh,
            "pwl": pwl,
        })
    return in_maps


def kernel(x, qkv_w, qkv_b, proj_w, proj_b, rel_pos_h, rel_pos_w):
    from concourse.bass_utils import run_bass_kernel_spmd

    x = np.asarray(x, dtype=np.float32)
    qkv_w = np.asarray(qkv_w, dtype=np.float32)
    qkv_b = np.asarray(qkv_b, dtype=np.float32)
    proj_w = np.asarray(proj_w, dtype=np.float32)
    proj_b = np.asarray(proj_b, dtype=np.float32)
    rel_pos_h = np.asarray(rel_pos_h, dtype=np.float32)
    rel_pos_w = np.asarray(rel_pos_w, dtype=np.float32)

    if "nc" not in _prog_cache:
        _prog_cache["nc"] = _build_program()
    nc = _prog_cache["nc"]

    in_maps = _host_inputs(x, qkv_w, qkv_b, proj_w, rel_pos_h, rel_pos_w)
    res = run_bass_kernel_spmd(nc, in_maps, core_ids=list(range(8)))

    out = np.zeros((N, C), dtype=np.float32)
    for core in range(8):
        g, j = core // QB, core % QB
        out[QL * j:QL * j + QL, :] += res.results[core]["out"].astype(np.float32).T / SW
    out += proj_b[None, :]
    return out.reshape(1, H, W, C).astype(np.float32)
